# revision 1
# baseline (speedup 1.0000x reference)
"""Trainium2 Bass kernel for a PointNet-style neighborhood encoder.

Computation (matches the reference nn.Module):
    h = relu(relu(relu(points @ W0 + b0) @ W1 + b1) @ W2 + b2)   # [N,3] -> [N,128]
    pooled = segment_max(h, cluster)                             # [C,128], 32 pts/cluster
    out = relu(relu(pooled @ G0 + g0) @ G1 + g1)                 # [C,256]

Sharding: data-parallel over points across 8 NeuronCores (cluster
boundaries are shard-aligned because clusters are contiguous, 32
points each). Weights are replicated. No collectives; the host
scatters inputs and gathers per-core outputs.

Device strategy (per core, n = 262144 points = 65536 quad-columns):
  - Host packs points feature-major, 4 points per 128-partition column
    ("quads"): pts4[3a+f, q] = points[4q+a, f], so layer 0 is a single
    block-diagonal matmul (K=12, M=128) producing h0 for 4 points/col.
  - Layer 1 uses two permuted block-diagonal stationaries W1A/W1B
    (K=128, M=128) producing h1 with 2 points per column.
  - Layer 2 uses W2 duplicated on both partition halves; 4 sub-matmuls
    (K=64, M=128) with rhs partition slices map to distinct PE row
    groups, producing z = W2^T h1 (bias/relu deferred) in PSUM.
  - segment_max: relu is monotone and b2 is constant per feature, so
    pooled = relu(max_p(z) + b2). max over (4 tensors x 8 quads) is ONE
    VectorE tensor_reduce(axis=XY) straight out of PSUM per sub-chunk.
  - ScalarE (ACT) does every relu+bias PSUM->SBUF evacuation; VectorE
    only does the pooling reduces. bf16 activations everywhere
    (PSUM stays f32 as the HW requires).
  - Global MLP on pooled [128, 8192] per core; output is written
    feature-major [256, 8192] bf16 and transposed/upcast on the host.
"""

import numpy as np

# ---- problem geometry (hardcoded per contract) ----
N = 2097152          # total points
C = 65536            # clusters
PTS = 32             # points per cluster
NCORES = 8
NPC = N // NCORES    # points per core = 262144
N4C = NPC // 4       # quad-columns per core = 65536
CPC = C // NCORES    # clusters per core = 8192

BIG = 1024           # quad-columns per big-chunk
SUB = 256            # quad-columns per L2/pool sub-chunk
NCHUNK = N4C // BIG  # 64
NSUB = BIG // SUB    # 4

_CACHE = {}


def _bf16():
    import ml_dtypes
    return ml_dtypes.bfloat16


def _build_module(n4c: int):
    """Build the Bass module (SPMD program, same for all cores)."""
    import concourse.bass as bass
    import concourse.bacc as bacc
    import concourse.tile as tile
    from concourse import mybir

    BF = mybir.dt.bfloat16
    F32 = mybir.dt.float32
    RELU = mybir.ActivationFunctionType.Relu
    MAX = mybir.AluOpType.max
    XY = mybir.AxisListType.XY

    nchunk = n4c // BIG
    cpc = n4c // 8          # clusters per core for this size

    nc = bacc.Bacc()

    # ---- DRAM I/O ----
    pts4 = nc.dram_tensor("pts4", [12, n4c], BF, kind="ExternalInput")
    w0q = nc.dram_tensor("w0q", [12, 128], BF, kind="ExternalInput")
    w1a = nc.dram_tensor("w1a", [128, 128], BF, kind="ExternalInput")
    w1b = nc.dram_tensor("w1b", [128, 128], BF, kind="ExternalInput")
    w2d = nc.dram_tensor("w2d", [128, 128], BF, kind="ExternalInput")
    g0w = nc.dram_tensor("g0w", [128, 128], BF, kind="ExternalInput")
    g1lo = nc.dram_tensor("g1lo", [128, 128], BF, kind="ExternalInput")
    g1hi = nc.dram_tensor("g1hi", [128, 128], BF, kind="ExternalInput")
    b0q = nc.dram_tensor("b0q", [128, 1], F32, kind="ExternalInput")
    b1d = nc.dram_tensor("b1d", [128, 1], F32, kind="ExternalInput")
    b2v = nc.dram_tensor("b2v", [128, 1], F32, kind="ExternalInput")
    g0v = nc.dram_tensor("g0v", [128, 1], F32, kind="ExternalInput")
    g1l = nc.dram_tensor("g1l", [128, 1], F32, kind="ExternalInput")
    g1h = nc.dram_tensor("g1h", [128, 1], F32, kind="ExternalInput")
    outt = nc.dram_tensor("outt", [256, cpc], BF, kind="ExternalOutput")

    from contextlib import ExitStack
    with tile.TileContext(nc) as tc, ExitStack() as ctx:
        singles = ctx.enter_context(tc.tile_pool(name="singles", bufs=1))
        ppts = ctx.enter_context(tc.tile_pool(name="ppts", bufs=3))
        ph0s = ctx.enter_context(tc.tile_pool(name="ph0s", bufs=2))
        ph1s = ctx.enter_context(tc.tile_pool(name="ph1s", bufs=2))
        psum_h = ctx.enter_context(tc.tile_pool(name="psum_h", bufs=2, space="PSUM"))
        psum_h1 = ctx.enter_context(tc.tile_pool(name="psum_h1", bufs=1, space="PSUM"))
        psum_z = ctx.enter_context(tc.tile_pool(name="psum_z", bufs=2, space="PSUM"))

        # ---- load constants ----
        w0q_s = singles.tile([12, 128], BF)
        w1a_s = singles.tile([128, 128], BF)
        w1b_s = singles.tile([128, 128], BF)
        w2d_s = singles.tile([128, 128], BF)
        g0w_s = singles.tile([128, 128], BF)
        g1lo_s = singles.tile([128, 128], BF)
        g1hi_s = singles.tile([128, 128], BF)
        for dst, src in ((w0q_s, w0q), (w1a_s, w1a), (w1b_s, w1b),
                         (w2d_s, w2d), (g0w_s, g0w), (g1lo_s, g1lo),
                         (g1hi_s, g1hi)):
            nc.gpsimd.dma_start(out=dst[:], in_=src[:])
        b0q_s = singles.tile([128, 1], F32)
        b1d_s = singles.tile([128, 1], F32)
        b2v_s = singles.tile([128, 1], F32)
        g0v_s = singles.tile([128, 1], F32)
        g1l_s = singles.tile([128, 1], F32)
        g1h_s = singles.tile([128, 1], F32)
        for dst, src in ((b0q_s, b0q), (b1d_s, b1d), (b2v_s, b2v),
                         (g0v_s, g0v), (g1l_s, g1l), (g1h_s, g1h)):
            nc.gpsimd.dma_start(out=dst[:], in_=src[:])

        # pooled max(z) accumulator for the whole core
        pooled = singles.tile([128, cpc], BF)

        # ---- main loop over point chunks (L2/pool shifted one chunk) ----
        def emit_l2_pair(i, jpair, h1as, h1bs):
            """L2 matmuls for subs (2*jpair, 2*jpair+1) of chunk i, weight-
            batched (both lo-half MMs, then both hi-half), then the pools."""
            zps = []
            for j in (2 * jpair, 2 * jpair + 1):
                s0 = j * SUB
                zp = psum_z.tile([128, 4 * SUB], F32, tag="zp")
                zps.append((j, s0, zp))
            for j, s0, zp in zps:
                nc.tensor.matmul(zp[:, 0:SUB],
                                 w2d_s[0:64, :], h1as[0:64, s0:s0 + SUB])
                nc.tensor.matmul(zp[:, SUB:2 * SUB],
                                 w2d_s[0:64, :], h1bs[0:64, s0:s0 + SUB])
            for j, s0, zp in zps:
                nc.tensor.matmul(zp[:, 2 * SUB:3 * SUB],
                                 w2d_s[64:128, :], h1as[64:128, s0:s0 + SUB])
                nc.tensor.matmul(zp[:, 3 * SUB:4 * SUB],
                                 w2d_s[64:128, :], h1bs[64:128, s0:s0 + SUB])
            for j, s0, zp in zps:
                # pooled_raw = max over (4 tensors x 8 quads) per cluster
                zv = zp.rearrange("p (t c q) -> p c t q", t=4, q=8)
                base = i * (BIG // 8) + j * (SUB // 8)
                nc.vector.tensor_reduce(
                    pooled[:, base:base + SUB // 8], zv, axis=XY, op=MAX)

        g0in = singles.tile([128, cpc], BF)
        g1in = singles.tile([128, cpc], BF)
        goutL = singles.tile([128, cpc], BF)
        goutH = singles.tile([128, cpc], BF)

        def g_task_g0(k, h):
            sl = slice(k * 512 + h * 256, k * 512 + (h + 1) * 256)
            nc.scalar.activation(g0in[:, sl], pooled[:, sl], RELU,
                                 bias=b2v_s[:])
            gp = psum_h.tile([128, 256], F32, tag="h0p")
            nc.tensor.matmul(gp[:], g0w_s[:], g0in[:, sl])
            nc.scalar.activation(g1in[:, sl], gp[:], RELU, bias=g0v_s[:])

        def g_task_lo(k, h):
            sl = slice(k * 512 + h * 256, k * 512 + (h + 1) * 256)
            gpl = psum_h.tile([128, 256], F32, tag="h0p")
            nc.tensor.matmul(gpl[:], g1lo_s[:], g1in[:, sl])
            nc.scalar.activation(goutL[:, sl], gpl[:], RELU, bias=g1l_s[:])
            nc.sync.dma_start(out=outt[0:128, sl], in_=goutL[:, sl])

        def g_task_hi(k, h):
            sl = slice(k * 512 + h * 256, k * 512 + (h + 1) * 256)
            gph = psum_h.tile([128, 256], F32, tag="h0p")
            nc.tensor.matmul(gph[:], g1hi_s[:], g1in[:, sl])
            nc.scalar.activation(goutH[:, sl], gph[:], RELU, bias=g1h_s[:])
            nc.sync.dma_start(out=outt[128:256, sl], in_=goutH[:, sl])

        def emit_g_block(k):
            for h in (0, 1):
                g_task_g0(k, h); g_task_lo(k, h); g_task_hi(k, h)

        g_tasks = []

        def pop_g_task():
            if g_tasks:
                fn, k, h = g_tasks.pop(0)
                fn(k, h)

        prev = None   # (i, h1as, h1bs) pending L2+pool
        for i in range(nchunk):
            c0 = i * BIG
            pts_t = ppts.tile([12, BIG], BF)
            nc.sync.dma_start(out=pts_t[:], in_=pts4[:, c0:c0 + BIG])

            # L0: 4-point block-diagonal matmul, K=12 -> M=128
            h0pa = psum_h.tile([128, 512], F32, tag="h0p")
            h0pb = psum_h.tile([128, 512], F32, tag="h0p")
            nc.tensor.matmul(h0pa[:], w0q_s[:], pts_t[:, 0:512])
            nc.tensor.matmul(h0pb[:], w0q_s[:], pts_t[:, 512:1024])
            h0s = ph0s.tile([128, BIG], BF)
            nc.scalar.activation(h0s[:, 0:512], h0pa[:], RELU, bias=b0q_s[:])
            nc.scalar.activation(h0s[:, 512:1024], h0pb[:], RELU, bias=b0q_s[:])

            if prev is not None:
                emit_l2_pair(prev[0], 0, prev[1], prev[2])
                pop_g_task()
                emit_l2_pair(prev[0], 1, prev[1], prev[2])

            # L1: two block-diagonal stationaries -> h1 (2 pts/col)
            h1p = psum_h1.tile([128, BIG], F32, tag="h1p")
            nc.tensor.matmul(h1p[:, 0:512], w1a_s[:], h0s[:, 0:512])
            nc.tensor.matmul(h1p[:, 512:1024], w1a_s[:], h0s[:, 512:1024])
            h1as = ph1s.tile([128, BIG], BF, tag="h1as")
            if i == 0:
                nc.scalar.activation(h1as[:, 0:512], h1p[:, 0:512], RELU,
                                     bias=b1d_s[:])
                nc.scalar.activation(h1as[:, 512:1024], h1p[:, 512:1024],
                                     RELU, bias=b1d_s[:])
            else:
                nc.scalar.activation(h1as[:], h1p[:], RELU, bias=b1d_s[:])

            if i == 0:
                # prologue: borrow an idle z slot so L1B needn't wait for
                # evac1A's PSUM release; the first L2 pair waits on h1bs
                # anyway, which releases the slot before L2 needs it.
                h1p2 = psum_z.tile([128, BIG], F32, tag="zp")
            else:
                h1p2 = psum_h1.tile([128, BIG], F32, tag="h1p")
            nc.tensor.matmul(h1p2[:, 0:512], w1b_s[:], h0s[:, 0:512])
            nc.tensor.matmul(h1p2[:, 512:1024], w1b_s[:], h0s[:, 512:1024])
            h1bs = ph1s.tile([128, BIG], BF, tag="h1bs")
            if i == 0:
                # split so the first L2 pair (cols 0:512) unblocks sooner
                nc.scalar.activation(h1bs[:, 0:512], h1p2[:, 0:512], RELU,
                                     bias=b1d_s[:])
                nc.scalar.activation(h1bs[:, 512:1024], h1p2[:, 512:1024],
                                     RELU, bias=b1d_s[:])
            else:
                nc.scalar.activation(h1bs[:], h1p2[:], RELU, bias=b1d_s[:])

            prev = (i, h1as, h1bs)

            # interleave global-MLP work once pooled slices complete:
            # block k (clusters 512k..512k+512) is pooled after iteration
            # 4k+4 starts (the shifted L2 of chunk 4k+3 was emitted above).
            if i >= 4 and (i - 4) % 4 == 0:
                k = (i - 4) // 4
                g_tasks.extend([(f, k, h) for h in (0, 1)
                                for f in (g_task_g0, g_task_lo, g_task_hi)])
            pop_g_task()

        # epilogue: overlap the final global-MLP work with the last pools.
        # Half-block (k, 0) only needs chunks 4k..4k+1 (pooled in-loop), so
        # everything except the very last half-block can run alongside the
        # final L2 pairs; (last_k, 1) needs the last chunk's pools.
        emit_l2_pair(prev[0], 0, prev[1], prev[2])
        for fn, k, h in g_tasks:
            fn(k, h)
        first_unpushed = ((nchunk - 5) // 4 + 1) if nchunk >= 5 else 0
        last_k = cpc // 512 - 1
        for k in range(first_unpushed, last_k + 1):
            for f in (g_task_g0, g_task_lo, g_task_hi):
                f(k, 0)
            if k < last_k:
                for f in (g_task_g0, g_task_lo, g_task_hi):
                    f(k, 1)
        emit_l2_pair(prev[0], 1, prev[1], prev[2])
        for f in (g_task_g0, g_task_lo, g_task_hi):
            f(last_k, 1)

    nc.compile()
    return nc


def _host_pack(points, W0, b0, W1, b1, W2, b2, G0, g0, G1, g1, n4c):
    """Build per-core input maps (host-side layout prep, numpy only)."""
    bf16 = _bf16()
    n = n4c * 4 * NCORES

    # pts4[3a+f, q] = points[4q+a, f]
    pts4 = np.ascontiguousarray(
        points[:n].reshape(-1, 4, 3).transpose(1, 2, 0).reshape(12, -1)
    ).astype(bf16)

    # W0 block-diagonal over 4 points: [12, 128]
    w0q = np.zeros((12, 128), np.float32)
    for a in range(4):
        w0q[3 * a:3 * a + 3, 32 * a:32 * a + 32] = W0
    # W1A/W1B: rows 32a+f; cols 64a'+g ; a' in {0,1} / {2,3}
    w1a = np.zeros((128, 128), np.float32)
    w1b = np.zeros((128, 128), np.float32)
    for a in range(2):
        w1a[32 * a:32 * a + 32, 64 * a:64 * a + 64] = W1
        w1b[32 * (a + 2):32 * (a + 2) + 32, 64 * a:64 * a + 64] = W1
    # W2 duplicated on both partition halves
    w2d = np.concatenate([W2, W2], axis=0)

    common = {
        "w0q": w0q.astype(bf16),
        "w1a": w1a.astype(bf16),
        "w1b": w1b.astype(bf16),
        "w2d": w2d.astype(bf16),
        "g0w": G0.astype(bf16),
        "g1lo": G1[:, :128].astype(bf16),
        "g1hi": G1[:, 128:].astype(bf16),
        "b0q": np.tile(b0, 4).reshape(128, 1).astype(np.float32),
        "b1d": np.tile(b1, 2).reshape(128, 1).astype(np.float32),
        "b2v": b2.reshape(128, 1).astype(np.float32),
        "g0v": g0.reshape(128, 1).astype(np.float32),
        "g1l": g1[:128].reshape(128, 1).astype(np.float32),
        "g1h": g1[128:].reshape(128, 1).astype(np.float32),
    }
    in_maps = []
    for c in range(NCORES):
        m = dict(common)
        m["pts4"] = np.ascontiguousarray(pts4[:, c * n4c:(c + 1) * n4c])
        in_maps.append(m)
    return in_maps


def _numpy_fallback(points, cluster, num_clusters,
                    W0, b0, W1, b1, W2, b2, G0, g0, G1, g1):
    h = points.astype(np.float32)
    for W, b in ((W0, b0), (W1, b1), (W2, b2)):
        h = np.maximum(h @ W + b, 0.0)
    order = np.argsort(cluster, kind="stable")
    cs = cluster[order]
    hs = h[order]
    starts = np.searchsorted(cs, np.arange(num_clusters), side="left")
    counts = np.bincount(cs, minlength=num_clusters)
    safe_starts = np.minimum(starts, max(len(hs) - 1, 0))
    seg = np.maximum.reduceat(hs, safe_starts, axis=0)
    seg[counts == 0] = -np.inf   # match segment_max identity on empties
    pooled = seg
    gx = pooled
    for W, b in ((G0, g0), (G1, g1)):
        gx = np.maximum(gx @ W + b, 0.0)
    return gx.astype(np.float32)


def kernel(**inputs) -> np.ndarray:
    points = np.asarray(inputs["points"], np.float32)
    cluster = np.asarray(inputs["cluster"]).astype(np.int64)
    num_clusters = int(np.asarray(inputs["num_clusters"]))
    W0 = np.asarray(inputs["W0"], np.float32); b0 = np.asarray(inputs["b0"], np.float32)
    W1 = np.asarray(inputs["W1"], np.float32); b1 = np.asarray(inputs["b1"], np.float32)
    W2 = np.asarray(inputs["W2"], np.float32); b2 = np.asarray(inputs["b2"], np.float32)
    G0 = np.asarray(inputs["G0"], np.float32); g0 = np.asarray(inputs["g0"], np.float32)
    G1 = np.asarray(inputs["G1"], np.float32); g1 = np.asarray(inputs["g1"], np.float32)

    expected = (points.shape == (N, 3) and num_clusters == C
                and cluster.shape == (N,))
    if expected:
        # contiguous equal clusters of 32 points, as produced by setup_inputs
        expected = bool(
            np.array_equal(cluster[::PTS], np.arange(C, dtype=np.int64))
            and np.array_equal(cluster, np.repeat(cluster[::PTS], PTS))
        )
    if not expected:
        return _numpy_fallback(points, cluster, num_clusters,
                               W0, b0, W1, b1, W2, b2, G0, g0, G1, g1)

    from concourse.bass_utils import run_bass_kernel_spmd

    if "nc" not in _CACHE:
        _CACHE["nc"] = _build_module(N4C)
    nc = _CACHE["nc"]

    in_maps = _host_pack(points, W0, b0, W1, b1, W2, b2, G0, g0, G1, g1, N4C)
    res = run_bass_kernel_spmd(nc, in_maps, core_ids=list(range(NCORES)))
    outs = []
    for c in range(NCORES):
        o = np.asarray(res.results[c]["outt"]).astype(np.float32)  # [256, CPC]
        outs.append(o.T)                                           # [CPC, 256]
    return np.ascontiguousarray(np.concatenate(outs, axis=0))



# revision 12
# speedup vs baseline: 1.0161x; 1.0161x over previous
"""Trainium2 Bass kernel for a PointNet-style neighborhood encoder.

Computation (matches the reference nn.Module):
    h = relu(relu(relu(points @ W0 + b0) @ W1 + b1) @ W2 + b2)   # [N,3] -> [N,128]
    pooled = segment_max(h, cluster)                             # [C,128], 32 pts/cluster
    out = relu(relu(pooled @ G0 + g0) @ G1 + g1)                 # [C,256]

Sharding: data-parallel over points across 8 NeuronCores (cluster
boundaries are shard-aligned because clusters are contiguous, 32
points each). Weights are replicated. No collectives; the host
scatters inputs and gathers per-core outputs.

Device strategy (per core, n = 262144 points = 65536 quad-columns):
  - Host packs points feature-major, 4 points per 128-partition column
    ("quads"): pts4[3a+f, q] = points[4q+a, f], so layer 0 is a single
    block-diagonal matmul (K=12, M=128) producing h0 for 4 points/col.
  - Layer 1 uses two permuted block-diagonal stationaries W1A/W1B
    (K=128, M=128) producing h1 with 2 points per column.
  - Layer 2 uses W2 duplicated on both partition halves; 4 sub-matmuls
    (K=64, M=128) with rhs partition slices map to distinct PE row
    groups, producing z = W2^T h1 (bias/relu deferred) in PSUM.
  - segment_max: relu is monotone and b2 is constant per feature, so
    pooled = relu(max_p(z) + b2). max over (4 tensors x 8 quads) is ONE
    VectorE tensor_reduce(axis=XY) straight out of PSUM per sub-chunk.
  - ScalarE (ACT) does every relu+bias PSUM->SBUF evacuation; VectorE
    only does the pooling reduces. bf16 activations everywhere
    (PSUM stays f32 as the HW requires).
  - Global MLP on pooled [128, 8192] per core; output is written
    feature-major [256, 8192] bf16 and transposed/upcast on the host.
"""

import numpy as np

# ---- problem geometry (hardcoded per contract) ----
N = 2097152          # total points
C = 65536            # clusters
PTS = 32             # points per cluster
NCORES = 8
NPC = N // NCORES    # points per core = 262144
N4C = NPC // 4       # quad-columns per core = 65536
CPC = C // NCORES    # clusters per core = 8192

BIG = 1024           # quad-columns per big-chunk
SUB = 256            # quad-columns per L2/pool sub-chunk
NCHUNK = N4C // BIG  # 64
NSUB = BIG // SUB    # 4

_CACHE = {}


def _bf16():
    import ml_dtypes
    return ml_dtypes.bfloat16


def _build_module(n4c: int):
    """Build the Bass module (SPMD program, same for all cores).

    Engine assignment per steady-state iteration (chunk of 1024 quad-cols
    = 4096 points = 128 clusters):
      - PE: L0 (2 mm), L1a/L1b (4 mm), L2 fills for chunk i-1 (16 mm into
        4 PSUM tiles zp_j laid out (t4, c32, q8)), plus one G matmul on a
        4-iteration cadence.
      - ACT: the three PSUM->SBUF relu+bias evacuations (h0s, h1as, h1bs),
        one 1024-col instruction each.
      - DVE: segment-max level 1: tensor_tensor max over each zp's t-halves
        (PSUM pair-read at 0.52ns/input col, vs 1.04 for tensor_reduce),
        plus tree level T2 (bf16 SBUF 2x mode) and the g0in bias+relu.
      - Pool (gpsimd): tree levels T1/T3/T4 on bf16 SBUF and the G-chain
        PSUM evacuations.
    The max tree: zp (t4,c,q8) --DVE--> s_all (t2,c,q8) --T1--> (c,q8)
    --T2--> (c,q4) --T3--> (c,q2) --T4--> pooled (c).  All maxes commute;
    bf16 rounding is monotone so round-then-max == max-then-round.
    """
    import concourse.bass as bass
    import concourse.bacc as bacc
    import concourse.tile as tile
    from concourse import mybir

    BF = mybir.dt.bfloat16
    F32 = mybir.dt.float32
    RELU = mybir.ActivationFunctionType.Relu
    MAX = mybir.AluOpType.max
    ADD = mybir.AluOpType.add
    XY = mybir.AxisListType.XY
    COPY = mybir.ActivationFunctionType.Copy

    nchunk = n4c // BIG
    cpc = n4c // 8          # clusters per core for this size
    nblk = cpc // 512       # global-MLP blocks of 512 clusters

    nc = bacc.Bacc()

    # ---- DRAM I/O ----
    pts4 = nc.dram_tensor("pts4", [12, n4c], BF, kind="ExternalInput")
    w0q = nc.dram_tensor("w0q", [12, 128], BF, kind="ExternalInput")
    w1a = nc.dram_tensor("w1a", [128, 128], BF, kind="ExternalInput")
    w1b = nc.dram_tensor("w1b", [128, 128], BF, kind="ExternalInput")
    w2d = nc.dram_tensor("w2d", [128, 128], BF, kind="ExternalInput")
    g0w = nc.dram_tensor("g0w", [128, 128], BF, kind="ExternalInput")
    g1lo = nc.dram_tensor("g1lo", [128, 128], BF, kind="ExternalInput")
    g1hi = nc.dram_tensor("g1hi", [128, 128], BF, kind="ExternalInput")
    b0q = nc.dram_tensor("b0q", [128, 1], F32, kind="ExternalInput")
    b1d = nc.dram_tensor("b1d", [128, 1], F32, kind="ExternalInput")
    b2v = nc.dram_tensor("b2v", [128, 1], F32, kind="ExternalInput")
    g0v = nc.dram_tensor("g0v", [128, 1], F32, kind="ExternalInput")
    g1l = nc.dram_tensor("g1l", [128, 1], F32, kind="ExternalInput")
    g1h = nc.dram_tensor("g1h", [128, 1], F32, kind="ExternalInput")
    outt = nc.dram_tensor("outt", [256, cpc], BF, kind="ExternalOutput")

    from contextlib import ExitStack
    with tile.TileContext(nc) as tc, ExitStack() as ctx:
        singles = ctx.enter_context(tc.tile_pool(name="singles", bufs=1))
        ppts = ctx.enter_context(tc.tile_pool(name="ppts", bufs=3))
        ph0s = ctx.enter_context(tc.tile_pool(name="ph0s", bufs=2))
        ph1s = ctx.enter_context(tc.tile_pool(name="ph1s", bufs=4))
        pzc = ctx.enter_context(tc.tile_pool(name="pzc", bufs=2))
        pu1 = ctx.enter_context(tc.tile_pool(name="pu1", bufs=2))
        pu2 = ctx.enter_context(tc.tile_pool(name="pu2", bufs=2))
        pu3 = ctx.enter_context(tc.tile_pool(name="pu3", bufs=2))
        pu4 = ctx.enter_context(tc.tile_pool(name="pu4", bufs=2))
        pg0 = ctx.enter_context(tc.tile_pool(name="pg0", bufs=2))
        pg1 = ctx.enter_context(tc.tile_pool(name="pg1", bufs=2))
        pgo = ctx.enter_context(tc.tile_pool(name="pgo", bufs=4))
        # PSUM: phx (2 bufs x 1024 f32 = 4 banks) rotates h0p -> [gp] ->
        # h1p -> h1p2; pz (2 bufs x 1024 = 4 banks) rotates the 4 zp tiles.
        phx = ctx.enter_context(tc.tile_pool(name="phx", bufs=2, space="PSUM"))
        pz = ctx.enter_context(tc.tile_pool(name="pz", bufs=2, space="PSUM"))

        # ---- load constants ----
        w0q_s = singles.tile([12, 128], BF)
        w1a_s = singles.tile([128, 128], BF)
        w1b_s = singles.tile([128, 128], BF)
        w2d_s = singles.tile([128, 128], BF)
        g0w_s = singles.tile([128, 128], BF)
        g1lo_s = singles.tile([128, 128], BF)
        g1hi_s = singles.tile([128, 128], BF)
        for dst, src in ((w0q_s, w0q), (w1a_s, w1a), (w1b_s, w1b),
                         (w2d_s, w2d), (g0w_s, g0w), (g1lo_s, g1lo),
                         (g1hi_s, g1hi)):
            nc.gpsimd.dma_start(out=dst[:], in_=src[:])
        b0q_s = singles.tile([128, 1], F32)
        b1d_s = singles.tile([128, 1], F32)
        b2v_s = singles.tile([128, 1], F32)
        g0v_s = singles.tile([128, 1], F32)
        g1l_s = singles.tile([128, 1], F32)
        g1h_s = singles.tile([128, 1], F32)
        for dst, src in ((b0q_s, b0q), (b1d_s, b1d), (b2v_s, b2v),
                         (g0v_s, g0v), (g1l_s, g1l), (g1h_s, g1h)):
            nc.gpsimd.dma_start(out=dst[:], in_=src[:])

        # pooled raw max(z) accumulator (pre-bias/relu), bf16
        pooled = singles.tile([128, cpc], BF)

        # ---- helpers ----
        st = {}       # chunk -> {"h1as","h1bs","s_all","t1","t2"}
        gst = {}      # block -> {"g0in","g1in"}

        def fills(c, j, zp, ts):
            """L2 matmuls (subset `ts` of the 4 t-slots) for sub-chunk j of
            chunk c into zp: layout (t4, c32, q8) per 256-col t-slot."""
            s0 = j * SUB
            h1as = st[c]["h1as"]; h1bs = st[c]["h1bs"]
            for t in ts:
                src, half = ((h1as, 0), (h1bs, 0), (h1as, 1), (h1bs, 1))[t]
                nc.tensor.matmul(zp[:, t * SUB:(t + 1) * SUB],
                                 w2d_s[half * 64:half * 64 + 64, :],
                                 src[half * 64:half * 64 + 64, s0:s0 + SUB])

        def reduce_zp(c, j, zp):
            """DVE tensor_reduce (t,q) straight out of PSUM -> 32 pooled."""
            zv = zp.rearrange("p (t c q) -> p c t q", t=4, q=8)
            base = c * 128 + j * 32
            nc.vector.tensor_reduce(pooled[:, base:base + 32], zv[:],
                                    axis=XY, op=MAX)

        # G-phase schedule: block k phases 0..3 at iterations 4k+9+p
        # (C-route pooled cols land one iteration later than R-route)
        gph = {}
        for k in range(nblk):
            for p in range(4):
                gph[4 * k + 9 + p] = (p, k)

        # Iteration `it` emits: L1a/L1b + h1 evacs for chunk it, L0 + h0s
        # for chunk it+1 (one iteration early, so the L0->h0s->L1a chain
        # spans an iteration boundary instead of serializing inside one),
        # L2 fills+merges for it-1, tree T1 for it-2, T2 for it-3, T3+T4
        # for it-4, and one G phase.
        for it in range(-1, nchunk + 9):
            mc = it if 0 <= it < nchunk else None       # main chunk
            lc = it + 1 if it + 1 < nchunk else None    # L0 chunk
            cf = it - 1 if 1 <= it - 1 + 1 <= nchunk else None
            c1 = it - 2 if 0 <= it - 2 < nchunk else None
            c2 = it - 3 if 0 <= it - 3 < nchunk else None
            c3 = it - 4 if 0 <= it - 4 < nchunk else None
            g = gph.get(it)

            # -- PE pos 1: L1a(mc) + ACT h1as --
            if mc is not None:
                h0s = st[mc]["h0s"]
                h1p = phx.tile([128, BIG], F32, tag="hx")
                nc.tensor.matmul(h1p[:, 0:512], w1a_s[:], h0s[:, 0:512])
                nc.tensor.matmul(h1p[:, 512:1024], w1a_s[:], h0s[:, 512:1024])
                h1as = ph1s.tile([128, BIG], BF, tag="h1as")
                nc.scalar.activation(h1as[:], h1p[:], RELU, bias=b1d_s[:])
                st[mc]["h1as"] = h1as

            # -- PE pos 2: L0(lc) + ACT h0s --
            if lc is not None:
                pts_t = ppts.tile([12, BIG], BF, tag="pts")
                nc.sync.dma_start(out=pts_t[:], in_=pts4[:, lc * BIG:(lc + 1) * BIG])
                h0p = phx.tile([128, BIG], F32, tag="hx")
                nc.tensor.matmul(h0p[:, 0:512], w0q_s[:], pts_t[:, 0:512])
                nc.tensor.matmul(h0p[:, 512:1024], w0q_s[:], pts_t[:, 512:1024])
                h0s_n = ph0s.tile([128, BIG], BF, tag="h0s")
                nc.scalar.activation(h0s_n[:], h0p[:], RELU, bias=b0q_s[:])
                st[lc] = {"h0s": h0s_n}

            # -- PE pos 3/4: fills j0, j1 + DVE reduces --
            if cf is not None:
                zp0 = pz.tile([128, BIG], F32, tag="zp")
                fills(cf, 0, zp0, (0, 2, 1, 3))
                zp1 = pz.tile([128, BIG], F32, tag="zp")
                fills(cf, 1, zp1, (0, 2, 1, 3))
                reduce_zp(cf, 0, zp0)
                reduce_zp(cf, 1, zp1)

            # -- PE pos 5: G matmul (phases 1..3) + Pool evac --
            if g is not None and g[0] >= 1:
                p, k = g
                sl = slice(k * 512, (k + 1) * 512)
                gp = phx.tile([128, 512], F32, tag="hx")
                if p == 1:
                    nc.tensor.matmul(gp[:], g0w_s[:], gst[k]["g0in"][:])
                    g1in = pg1.tile([128, 512], BF, tag="g1in")
                    nc.vector.tensor_scalar(out=g1in[:], in0=gp[:],
                                            scalar1=g0v_s[:], scalar2=0.0,
                                            op0=ADD, op1=MAX)
                    gst[k]["g1in"] = g1in
                else:
                    wsrc, bsrc, base = ((g1lo_s, g1l_s, 0),
                                        (g1hi_s, g1h_s, 128))[p - 2]
                    nc.tensor.matmul(gp[:], wsrc[:], gst[k]["g1in"][:])
                    go = pgo.tile([128, 512], BF, tag="gout")
                    nc.scalar.activation(go[:], gp[:], RELU, bias=bsrc[:])
                    nc.sync.dma_start(out=outt[base:base + 128, sl], in_=go[:])
            else:
                # parity keeper: unused phx acquisition so the 4-slot
                # rotation pairing stays identical on non-Gmm iterations
                gp = phx.tile([128, 512], F32, tag="hx")

            # -- PE pos 7: L1b(mc) + ACT h1bs --
            if mc is not None:
                h1p2 = phx.tile([128, BIG], F32, tag="hx")
                nc.tensor.matmul(h1p2[:, 0:512], w1b_s[:], h0s[:, 0:512])
                nc.tensor.matmul(h1p2[:, 512:1024], w1b_s[:], h0s[:, 512:1024])
                h1bs = ph1s.tile([128, BIG], BF, tag="h1bs")
                nc.scalar.activation(h1bs[:], h1p2[:], RELU, bias=b1d_s[:])
                st[mc]["h1bs"] = h1bs

            # -- PE pos 6: fills j2 + reduce --
            if cf is not None:
                zp2 = pz.tile([128, BIG], F32, tag="zp")
                fills(cf, 2, zp2, (0, 2, 1, 3))
                reduce_zp(cf, 2, zp2)

            # -- PE pos 8: fills j3; route R (every 4th chunk) or C --
            if cf is not None:
                zp3 = pz.tile([128, BIG], F32, tag="zp")
                fills(cf, 3, zp3, (0, 2, 1, 3))
                if False:
                    reduce_zp(cf, 3, zp3)
                else:
                    zc = pzc.tile([128, BIG], BF, tag="zc")
                    nc.scalar.activation(zc[:], zp3[:], COPY, bias=0.0)
                    st[cf]["zc"] = zc

            # -- DVE tail: C-route tt-max tree for chunk c1's zp3 copy --
            if c1 is not None:
                zc = st[c1]["zc"]
                u1 = pu1.tile([128, 512], BF, tag="u1")
                nc.vector.tensor_tensor(u1[:], zc[:, 0:512], zc[:, 512:1024],
                                        op=MAX)
                u2 = pu2.tile([128, 256], BF, tag="u2")
                nc.vector.tensor_tensor(u2[:], u1[:, 0:256], u1[:, 256:512],
                                        op=MAX)
                u3 = pu3.tile([128, 128], BF, tag="u3")
                v3 = u2.rearrange("p (c two q) -> p c two q", two=2, q=4)
                nc.vector.tensor_tensor(
                    u3.rearrange("p (c q) -> p c q", q=4)[:],
                    v3[:, :, 0, :], v3[:, :, 1, :], op=MAX)
                u4 = pu4.tile([128, 64], BF, tag="u4")
                v4 = u3.rearrange("p (c two q) -> p c two q", two=2, q=2)
                nc.vector.tensor_tensor(
                    u4.rearrange("p (c q) -> p c q", q=2)[:],
                    v4[:, :, 0, :], v4[:, :, 1, :], op=MAX)
                v5 = u4.rearrange("p (c two) -> p c two", two=2)
                nc.vector.tensor_tensor(
                    pooled[:, c1 * 128 + 96:c1 * 128 + 128]
                    .rearrange("p (c one) -> p c one", one=1)[:],
                    v5[:, :, 0:1], v5[:, :, 1:2], op=MAX)
            if g is not None and g[0] == 0:
                k = g[1]
                g0in = pg0.tile([128, 512], BF, tag="g0in")
                nc.gpsimd.tensor_scalar(out=g0in[:],
                                        in0=pooled[:, k * 512:(k + 1) * 512],
                                        scalar1=b2v_s[:], scalar2=0.0,
                                        op0=ADD, op1=MAX)
                gst[k] = {"g0in": g0in}

    nc.compile()
    return nc


def _host_pack(points, W0, b0, W1, b1, W2, b2, G0, g0, G1, g1, n4c):
    """Build per-core input maps (host-side layout prep, numpy only)."""
    bf16 = _bf16()
    n = n4c * 4 * NCORES

    # pts4[3a+f, q] = points[4q+a, f]
    pts4 = np.ascontiguousarray(
        points[:n].reshape(-1, 4, 3).transpose(1, 2, 0).reshape(12, -1)
    ).astype(bf16)

    # W0 block-diagonal over 4 points: [12, 128]
    w0q = np.zeros((12, 128), np.float32)
    for a in range(4):
        w0q[3 * a:3 * a + 3, 32 * a:32 * a + 32] = W0
    # W1A/W1B: rows 32a+f; cols 64a'+g ; a' in {0,1} / {2,3}
    w1a = np.zeros((128, 128), np.float32)
    w1b = np.zeros((128, 128), np.float32)
    for a in range(2):
        w1a[32 * a:32 * a + 32, 64 * a:64 * a + 64] = W1
        w1b[32 * (a + 2):32 * (a + 2) + 32, 64 * a:64 * a + 64] = W1
    # W2 duplicated on both partition halves
    w2d = np.concatenate([W2, W2], axis=0)

    common = {
        "w0q": w0q.astype(bf16),
        "w1a": w1a.astype(bf16),
        "w1b": w1b.astype(bf16),
        "w2d": w2d.astype(bf16),
        "g0w": G0.astype(bf16),
        "g1lo": G1[:, :128].astype(bf16),
        "g1hi": G1[:, 128:].astype(bf16),
        "b0q": np.tile(b0, 4).reshape(128, 1).astype(np.float32),
        "b1d": np.tile(b1, 2).reshape(128, 1).astype(np.float32),
        "b2v": b2.reshape(128, 1).astype(np.float32),
        "g0v": g0.reshape(128, 1).astype(np.float32),
        "g1l": g1[:128].reshape(128, 1).astype(np.float32),
        "g1h": g1[128:].reshape(128, 1).astype(np.float32),
    }
    in_maps = []
    for c in range(NCORES):
        m = dict(common)
        m["pts4"] = np.ascontiguousarray(pts4[:, c * n4c:(c + 1) * n4c])
        in_maps.append(m)
    return in_maps


def _numpy_fallback(points, cluster, num_clusters,
                    W0, b0, W1, b1, W2, b2, G0, g0, G1, g1):
    h = points.astype(np.float32)
    for W, b in ((W0, b0), (W1, b1), (W2, b2)):
        h = np.maximum(h @ W + b, 0.0)
    order = np.argsort(cluster, kind="stable")
    cs = cluster[order]
    hs = h[order]
    starts = np.searchsorted(cs, np.arange(num_clusters), side="left")
    counts = np.bincount(cs, minlength=num_clusters)
    safe_starts = np.minimum(starts, max(len(hs) - 1, 0))
    seg = np.maximum.reduceat(hs, safe_starts, axis=0)
    seg[counts == 0] = -np.inf   # match segment_max identity on empties
    pooled = seg
    gx = pooled
    for W, b in ((G0, g0), (G1, g1)):
        gx = np.maximum(gx @ W + b, 0.0)
    return gx.astype(np.float32)


def kernel(**inputs) -> np.ndarray:
    points = np.asarray(inputs["points"], np.float32)
    cluster = np.asarray(inputs["cluster"]).astype(np.int64)
    num_clusters = int(np.asarray(inputs["num_clusters"]))
    W0 = np.asarray(inputs["W0"], np.float32); b0 = np.asarray(inputs["b0"], np.float32)
    W1 = np.asarray(inputs["W1"], np.float32); b1 = np.asarray(inputs["b1"], np.float32)
    W2 = np.asarray(inputs["W2"], np.float32); b2 = np.asarray(inputs["b2"], np.float32)
    G0 = np.asarray(inputs["G0"], np.float32); g0 = np.asarray(inputs["g0"], np.float32)
    G1 = np.asarray(inputs["G1"], np.float32); g1 = np.asarray(inputs["g1"], np.float32)

    expected = (points.shape == (N, 3) and num_clusters == C
                and cluster.shape == (N,))
    if expected:
        # contiguous equal clusters of 32 points, as produced by setup_inputs
        expected = bool(
            np.array_equal(cluster[::PTS], np.arange(C, dtype=np.int64))
            and np.array_equal(cluster, np.repeat(cluster[::PTS], PTS))
        )
    if not expected:
        return _numpy_fallback(points, cluster, num_clusters,
                               W0, b0, W1, b1, W2, b2, G0, g0, G1, g1)

    from concourse.bass_utils import run_bass_kernel_spmd

    if "nc" not in _CACHE:
        _CACHE["nc"] = _build_module(N4C)
    nc = _CACHE["nc"]

    in_maps = _host_pack(points, W0, b0, W1, b1, W2, b2, G0, g0, G1, g1, N4C)
    res = run_bass_kernel_spmd(nc, in_maps, core_ids=list(range(NCORES)))
    outs = []
    for c in range(NCORES):
        o = np.asarray(res.results[c]["outt"]).astype(np.float32)  # [256, CPC]
        outs.append(o.T)                                           # [CPC, 256]
    return np.ascontiguousarray(np.concatenate(outs, axis=0))



# revision 18
# speedup vs baseline: 1.0223x; 1.0061x over previous
"""Trainium2 Bass kernel for a PointNet-style neighborhood encoder.

Computation (matches the reference nn.Module):
    h = relu(relu(relu(points @ W0 + b0) @ W1 + b1) @ W2 + b2)   # [N,3] -> [N,128]
    pooled = segment_max(h, cluster)                             # [C,128], 32 pts/cluster
    out = relu(relu(pooled @ G0 + g0) @ G1 + g1)                 # [C,256]

Sharding: data-parallel over points across 8 NeuronCores (cluster
boundaries are shard-aligned because clusters are contiguous, 32
points each). Weights are replicated. No collectives; the host
scatters inputs and gathers per-core outputs.

Device strategy (per core, n = 262144 points = 65536 quad-columns):
  - Host packs points feature-major, 4 points per 128-partition column
    ("quads"): pts4[3a+f, q] = points[4q+a, f], so layer 0 is a single
    block-diagonal matmul (K=12, M=128) producing h0 for 4 points/col.
  - Layer 1 uses two permuted block-diagonal stationaries W1A/W1B
    (K=128, M=128) producing h1 with 2 points per column.
  - Layer 2 uses W2 duplicated on both partition halves; 4 sub-matmuls
    (K=64, M=128) with rhs partition slices map to distinct PE row
    groups, producing z = W2^T h1 (bias/relu deferred) in PSUM.
  - segment_max: relu is monotone and b2 is constant per feature, so
    pooled = relu(max_p(z) + b2). max over (4 tensors x 8 quads) is ONE
    VectorE tensor_reduce(axis=XY) straight out of PSUM per sub-chunk.
  - ScalarE (ACT) does every relu+bias PSUM->SBUF evacuation; VectorE
    only does the pooling reduces. bf16 activations everywhere
    (PSUM stays f32 as the HW requires).
  - Global MLP on pooled [128, 8192] per core; output is written
    feature-major [256, 8192] bf16 and transposed/upcast on the host.
"""

import numpy as np

# ---- problem geometry (hardcoded per contract) ----
N = 2097152          # total points
C = 65536            # clusters
PTS = 32             # points per cluster
NCORES = 8
NPC = N // NCORES    # points per core = 262144
N4C = NPC // 4       # quad-columns per core = 65536
CPC = C // NCORES    # clusters per core = 8192

BIG = 1024           # quad-columns per big-chunk
SUB = 256            # quad-columns per L2/pool sub-chunk
NCHUNK = N4C // BIG  # 64
NSUB = BIG // SUB    # 4

_CACHE = {}


def _bf16():
    import ml_dtypes
    return ml_dtypes.bfloat16


def _build_module(n4c: int):
    """Build the Bass module (SPMD program, same for all cores).

    Engine assignment per steady-state iteration (chunk of 1024 quad-cols
    = 4096 points = 128 clusters).  Hardware legality constraints (the
    real walrus birverifier, stricter than CoreSim): at most one PSUM
    operand per instruction, Pool/gpsimd has no tensor_tensor and no PSUM
    access, DMA cannot read PSUM.
      - PE: L0 for chunk i+1 (emitted one iteration early so the
        L0->h0s->L1a chain spans an iteration boundary), L1a/L1b for
        chunk i, L2 fills for chunk i-1 (16 mm into 4 PSUM tiles zp_j
        laid out (t4, c32, q8)), plus one G matmul on a 4-iter cadence.
      - ACT: the three PSUM->SBUF relu+bias evacuations (h1as, h0s-next,
        h1bs; one 1024-col instruction each), the raw bf16 copy of zp3
        (route C), and the G-chain gout evacuations.
      - DVE: tensor_reduce (t,q)->cluster straight from PSUM for
        zp0..zp2 (route R), the 5-level bf16 2x tensor_tensor max tree
        over the copied zp3, and the g1in evacuation.
      - Pool (gpsimd): only g0in = relu(pooled + b2) (SBUF tensor_scalar).
    All maxes commute; bf16 rounding is monotone so round-then-max ==
    max-then-round, and relu/bias commute with max (applied post-pool).
    """
    import concourse.bass as bass
    import concourse.bacc as bacc
    import concourse.tile as tile
    from concourse import mybir

    BF = mybir.dt.bfloat16
    F32 = mybir.dt.float32
    RELU = mybir.ActivationFunctionType.Relu
    MAX = mybir.AluOpType.max
    ADD = mybir.AluOpType.add
    XY = mybir.AxisListType.XY
    COPY = mybir.ActivationFunctionType.Copy

    nchunk = n4c // BIG
    cpc = n4c // 8          # clusters per core for this size
    nblk = cpc // 512       # global-MLP blocks of 512 clusters

    nc = bacc.Bacc()

    # ---- DRAM I/O ----
    pts4 = nc.dram_tensor("pts4", [12, n4c], BF, kind="ExternalInput")
    w0q = nc.dram_tensor("w0q", [12, 128], BF, kind="ExternalInput")
    w1a = nc.dram_tensor("w1a", [128, 128], BF, kind="ExternalInput")
    w1b = nc.dram_tensor("w1b", [128, 128], BF, kind="ExternalInput")
    w2d = nc.dram_tensor("w2d", [128, 128], BF, kind="ExternalInput")
    g0w = nc.dram_tensor("g0w", [128, 128], BF, kind="ExternalInput")
    g1lo = nc.dram_tensor("g1lo", [128, 128], BF, kind="ExternalInput")
    g1hi = nc.dram_tensor("g1hi", [128, 128], BF, kind="ExternalInput")
    b0q = nc.dram_tensor("b0q", [128, 1], F32, kind="ExternalInput")
    b1d = nc.dram_tensor("b1d", [128, 1], F32, kind="ExternalInput")
    b2v = nc.dram_tensor("b2v", [128, 1], F32, kind="ExternalInput")
    g0v = nc.dram_tensor("g0v", [128, 1], F32, kind="ExternalInput")
    g1l = nc.dram_tensor("g1l", [128, 1], F32, kind="ExternalInput")
    g1h = nc.dram_tensor("g1h", [128, 1], F32, kind="ExternalInput")
    outt = nc.dram_tensor("outt", [256, cpc], BF, kind="ExternalOutput")

    from contextlib import ExitStack
    with tile.TileContext(nc) as tc, ExitStack() as ctx:
        singles = ctx.enter_context(tc.tile_pool(name="singles", bufs=1))
        ppts = ctx.enter_context(tc.tile_pool(name="ppts", bufs=3))
        ph0s = ctx.enter_context(tc.tile_pool(name="ph0s", bufs=2))
        ph1s = ctx.enter_context(tc.tile_pool(name="ph1s", bufs=4))
        pzc = ctx.enter_context(tc.tile_pool(name="pzc", bufs=2))
        pu1 = ctx.enter_context(tc.tile_pool(name="pu1", bufs=2))
        pu2 = ctx.enter_context(tc.tile_pool(name="pu2", bufs=2))
        pu3 = ctx.enter_context(tc.tile_pool(name="pu3", bufs=2))
        pu4 = ctx.enter_context(tc.tile_pool(name="pu4", bufs=2))
        pg0 = ctx.enter_context(tc.tile_pool(name="pg0", bufs=2))
        pg1 = ctx.enter_context(tc.tile_pool(name="pg1", bufs=2))
        pgo = ctx.enter_context(tc.tile_pool(name="pgo", bufs=4))
        # PSUM: phx (2 bufs x 1024 f32 = 4 banks) rotates h0p -> [gp] ->
        # h1p -> h1p2; pz (2 bufs x 1024 = 4 banks) rotates the 4 zp tiles.
        phx = ctx.enter_context(tc.tile_pool(name="phx", bufs=2, space="PSUM"))
        pz = ctx.enter_context(tc.tile_pool(name="pz", bufs=2, space="PSUM"))

        # ---- load constants ----
        w0q_s = singles.tile([12, 128], BF)
        w1a_s = singles.tile([128, 128], BF)
        w1b_s = singles.tile([128, 128], BF)
        w2d_s = singles.tile([128, 128], BF)
        g0w_s = singles.tile([128, 128], BF)
        g1lo_s = singles.tile([128, 128], BF)
        g1hi_s = singles.tile([128, 128], BF)
        for dst, src in ((w0q_s, w0q), (w1a_s, w1a), (w1b_s, w1b),
                         (w2d_s, w2d), (g0w_s, g0w), (g1lo_s, g1lo),
                         (g1hi_s, g1hi)):
            nc.gpsimd.dma_start(out=dst[:], in_=src[:])
        b0q_s = singles.tile([128, 1], F32)
        b1d_s = singles.tile([128, 1], F32)
        b2v_s = singles.tile([128, 1], F32)
        g0v_s = singles.tile([128, 1], F32)
        g1l_s = singles.tile([128, 1], F32)
        g1h_s = singles.tile([128, 1], F32)
        for dst, src in ((b0q_s, b0q), (b1d_s, b1d), (b2v_s, b2v),
                         (g0v_s, g0v), (g1l_s, g1l), (g1h_s, g1h)):
            nc.gpsimd.dma_start(out=dst[:], in_=src[:])

        # pooled raw max(z) accumulator (pre-bias/relu), bf16
        pooled = singles.tile([128, cpc], BF)

        # ---- helpers ----
        st = {}       # chunk -> {"h1as","h1bs","s_all","t1","t2"}
        gst = {}      # block -> {"g0in","g1in"}

        def fills(c, j, zp, ts):
            """L2 matmuls (subset `ts` of the 4 t-slots) for sub-chunk j of
            chunk c into zp: layout (t4, c32, q8) per 256-col t-slot."""
            s0 = j * SUB
            h1as = st[c]["h1as"]; h1bs = st[c]["h1bs"]
            for t in ts:
                src, half = ((h1as, 0), (h1bs, 0), (h1as, 1), (h1bs, 1))[t]
                nc.tensor.matmul(zp[:, t * SUB:(t + 1) * SUB],
                                 w2d_s[half * 64:half * 64 + 64, :],
                                 src[half * 64:half * 64 + 64, s0:s0 + SUB])

        def reduce_zp(c, j, zp):
            """DVE tensor_reduce (t,q) straight out of PSUM -> 32 pooled."""
            zv = zp.rearrange("p (t c q) -> p c t q", t=4, q=8)
            base = c * 128 + j * 32
            nc.vector.tensor_reduce(pooled[:, base:base + 32], zv[:],
                                    axis=XY, op=MAX)

        # G-phase schedule: block k phases 0..3 at iterations 4k+6+p
        # (block k's last pooled cols land in iteration 4k+5: chunk 4k+3's
        # C-route tree runs at c1 = it-2)
        gph = {}
        for k in range(nblk):
            for p in range(4):
                gph[4 * k + 9 + p] = (p, k)

        # Iteration `it` emits: L1a/L1b + h1 evacs for chunk it, L0 + h0s
        # for chunk it+1 (one iteration early, so the L0->h0s->L1a chain
        # spans an iteration boundary instead of serializing inside one),
        # L2 fills+merges for it-1, tree T1 for it-2, T2 for it-3, T3+T4
        # for it-4, and one G phase.
        for it in range(-1, nchunk + 9):
            mc = it if 0 <= it < nchunk else None       # main chunk
            lc = it + 1 if it + 1 < nchunk else None    # L0 chunk
            cf = it - 1 if 1 <= it - 1 + 1 <= nchunk else None
            c1 = it - 2 if 0 <= it - 2 < nchunk else None
            c2 = it - 3 if 0 <= it - 3 < nchunk else None
            c3 = it - 4 if 0 <= it - 4 < nchunk else None
            g = gph.get(it)

            # -- PE pos 1: L1a(mc) + ACT h1as --
            if mc is not None:
                h0s = st[mc]["h0s"]
                h1p = phx.tile([128, BIG], F32, tag="hx")
                nc.tensor.matmul(h1p[:, 0:512], w1a_s[:], h0s[:, 0:512])
                nc.tensor.matmul(h1p[:, 512:1024], w1a_s[:], h0s[:, 512:1024])
                h1as = ph1s.tile([128, BIG], BF, tag="h1as")
                nc.scalar.activation(h1as[:], h1p[:], RELU, bias=b1d_s[:])
                st[mc]["h1as"] = h1as

            # -- PE pos 2: L0(lc) + ACT h0s --
            if lc is not None:
                pts_t = ppts.tile([12, BIG], BF, tag="pts")
                nc.sync.dma_start(out=pts_t[:], in_=pts4[:, lc * BIG:(lc + 1) * BIG])
                h0p = phx.tile([128, BIG], F32, tag="hx")
                nc.tensor.matmul(h0p[:, 0:512], w0q_s[:], pts_t[:, 0:512])
                nc.tensor.matmul(h0p[:, 512:1024], w0q_s[:], pts_t[:, 512:1024])
                h0s_n = ph0s.tile([128, BIG], BF, tag="h0s")
                nc.scalar.activation(h0s_n[:], h0p[:], RELU, bias=b0q_s[:])
                st[lc] = {"h0s": h0s_n}

            # -- PE pos 3/4: fills j0, j1 + DVE reduces --
            if cf is not None:
                zp0 = pz.tile([128, BIG], F32, tag="zp")
                fills(cf, 0, zp0, (0, 2, 1, 3))
                zp1 = pz.tile([128, BIG], F32, tag="zp")
                fills(cf, 1, zp1, (0, 2, 1, 3))
                reduce_zp(cf, 0, zp0)
                reduce_zp(cf, 1, zp1)

            # -- PE pos 5: G matmul (phases 1..3) + Pool evac --
            if g is not None and g[0] >= 1:
                p, k = g
                sl = slice(k * 512, (k + 1) * 512)
                gp = phx.tile([128, 512], F32, tag="hx")
                if p == 1:
                    nc.tensor.matmul(gp[:], g0w_s[:], gst[k]["g0in"][:])
                    g1in = pg1.tile([128, 512], BF, tag="g1in")
                    if k % 2 == 0:
                        nc.vector.tensor_scalar(out=g1in[:], in0=gp[:],
                                                scalar1=g0v_s[:], scalar2=0.0,
                                                op0=ADD, op1=MAX)
                    else:
                        nc.scalar.activation(g1in[:], gp[:], RELU,
                                             bias=g0v_s[:])
                    gst[k]["g1in"] = g1in
                else:
                    wsrc, bsrc, base = ((g1lo_s, g1l_s, 0),
                                        (g1hi_s, g1h_s, 128))[p - 2]
                    nc.tensor.matmul(gp[:], wsrc[:], gst[k]["g1in"][:])
                    go = pgo.tile([128, 512], BF, tag="gout")
                    nc.scalar.activation(go[:], gp[:], RELU, bias=bsrc[:])
                    nc.sync.dma_start(out=outt[base:base + 128, sl], in_=go[:])
            else:
                # parity keeper: unused phx acquisition so the 4-slot
                # rotation pairing stays identical on non-Gmm iterations
                gp = phx.tile([128, 512], F32, tag="hx")

            # -- PE pos 7: L1b(mc) + ACT h1bs --
            if mc is not None:
                h1p2 = phx.tile([128, BIG], F32, tag="hx")
                nc.tensor.matmul(h1p2[:, 0:512], w1b_s[:], h0s[:, 0:512])
                nc.tensor.matmul(h1p2[:, 512:1024], w1b_s[:], h0s[:, 512:1024])
                h1bs = ph1s.tile([128, BIG], BF, tag="h1bs")
                nc.scalar.activation(h1bs[:], h1p2[:], RELU, bias=b1d_s[:])
                st[mc]["h1bs"] = h1bs

            # -- PE pos 6: fills j2 + reduce --
            if cf is not None:
                zp2 = pz.tile([128, BIG], F32, tag="zp")
                fills(cf, 2, zp2, (0, 2, 1, 3))
                reduce_zp(cf, 2, zp2)

            # -- PE pos 8: fills j3; route R (every 4th chunk) or C --
            if cf is not None:
                zp3 = pz.tile([128, BIG], F32, tag="zp")
                fills(cf, 3, zp3, (0, 2, 1, 3))
                if False:
                    reduce_zp(cf, 3, zp3)
                else:
                    zc = pzc.tile([128, BIG], BF, tag="zc")
                    nc.scalar.activation(zc[:], zp3[:], COPY, bias=0.0)
                    st[cf]["zc"] = zc

            # -- DVE tail: C-route tt-max tree for chunk c1's zp3 copy --
            if c1 is not None:
                zc = st[c1]["zc"]
                u1 = pu1.tile([128, 512], BF, tag="u1")
                nc.vector.tensor_tensor(u1[:], zc[:, 0:512], zc[:, 512:1024],
                                        op=MAX)
                u2 = pu2.tile([128, 256], BF, tag="u2")
                nc.vector.tensor_tensor(u2[:], u1[:, 0:256], u1[:, 256:512],
                                        op=MAX)
                u3 = pu3.tile([128, 128], BF, tag="u3")
                v3 = u2.rearrange("p (c two q) -> p c two q", two=2, q=4)
                nc.vector.tensor_tensor(
                    u3.rearrange("p (c q) -> p c q", q=4)[:],
                    v3[:, :, 0, :], v3[:, :, 1, :], op=MAX)
                u4 = pu4.tile([128, 64], BF, tag="u4")
                v4 = u3.rearrange("p (c two q) -> p c two q", two=2, q=2)
                nc.vector.tensor_tensor(
                    u4.rearrange("p (c q) -> p c q", q=2)[:],
                    v4[:, :, 0, :], v4[:, :, 1, :], op=MAX)
                v5 = u4.rearrange("p (c two) -> p c two", two=2)
                nc.vector.tensor_tensor(
                    pooled[:, c1 * 128 + 96:c1 * 128 + 128]
                    .rearrange("p (c one) -> p c one", one=1)[:],
                    v5[:, :, 0:1], v5[:, :, 1:2], op=MAX)
            if g is not None and g[0] == 0:
                k = g[1]
                g0in = pg0.tile([128, 512], BF, tag="g0in")
                nc.gpsimd.tensor_scalar(out=g0in[:],
                                        in0=pooled[:, k * 512:(k + 1) * 512],
                                        scalar1=b2v_s[:], scalar2=0.0,
                                        op0=ADD, op1=MAX)
                gst[k] = {"g0in": g0in}

    nc.compile()
    return nc


def _host_pack(points, W0, b0, W1, b1, W2, b2, G0, g0, G1, g1, n4c):
    """Build per-core input maps (host-side layout prep, numpy only)."""
    bf16 = _bf16()
    n = n4c * 4 * NCORES

    # pts4[3a+f, q] = points[4q+a, f]
    pts4 = np.ascontiguousarray(
        points[:n].reshape(-1, 4, 3).transpose(1, 2, 0).reshape(12, -1)
    ).astype(bf16)

    # W0 block-diagonal over 4 points: [12, 128]
    w0q = np.zeros((12, 128), np.float32)
    for a in range(4):
        w0q[3 * a:3 * a + 3, 32 * a:32 * a + 32] = W0
    # W1A/W1B: rows 32a+f; cols 64a'+g ; a' in {0,1} / {2,3}
    w1a = np.zeros((128, 128), np.float32)
    w1b = np.zeros((128, 128), np.float32)
    for a in range(2):
        w1a[32 * a:32 * a + 32, 64 * a:64 * a + 64] = W1
        w1b[32 * (a + 2):32 * (a + 2) + 32, 64 * a:64 * a + 64] = W1
    # W2 duplicated on both partition halves
    w2d = np.concatenate([W2, W2], axis=0)

    common = {
        "w0q": w0q.astype(bf16),
        "w1a": w1a.astype(bf16),
        "w1b": w1b.astype(bf16),
        "w2d": w2d.astype(bf16),
        "g0w": G0.astype(bf16),
        "g1lo": G1[:, :128].astype(bf16),
        "g1hi": G1[:, 128:].astype(bf16),
        "b0q": np.tile(b0, 4).reshape(128, 1).astype(np.float32),
        "b1d": np.tile(b1, 2).reshape(128, 1).astype(np.float32),
        "b2v": b2.reshape(128, 1).astype(np.float32),
        "g0v": g0.reshape(128, 1).astype(np.float32),
        "g1l": g1[:128].reshape(128, 1).astype(np.float32),
        "g1h": g1[128:].reshape(128, 1).astype(np.float32),
    }
    in_maps = []
    for c in range(NCORES):
        m = dict(common)
        m["pts4"] = np.ascontiguousarray(pts4[:, c * n4c:(c + 1) * n4c])
        in_maps.append(m)
    return in_maps


def _numpy_fallback(points, cluster, num_clusters,
                    W0, b0, W1, b1, W2, b2, G0, g0, G1, g1):
    h = points.astype(np.float32)
    for W, b in ((W0, b0), (W1, b1), (W2, b2)):
        h = np.maximum(h @ W + b, 0.0)
    order = np.argsort(cluster, kind="stable")
    cs = cluster[order]
    hs = h[order]
    starts = np.searchsorted(cs, np.arange(num_clusters), side="left")
    counts = np.bincount(cs, minlength=num_clusters)
    safe_starts = np.minimum(starts, max(len(hs) - 1, 0))
    seg = np.maximum.reduceat(hs, safe_starts, axis=0)
    seg[counts == 0] = -np.inf   # match segment_max identity on empties
    pooled = seg
    gx = pooled
    for W, b in ((G0, g0), (G1, g1)):
        gx = np.maximum(gx @ W + b, 0.0)
    return gx.astype(np.float32)


def kernel(**inputs) -> np.ndarray:
    points = np.asarray(inputs["points"], np.float32)
    cluster = np.asarray(inputs["cluster"]).astype(np.int64)
    num_clusters = int(np.asarray(inputs["num_clusters"]))
    W0 = np.asarray(inputs["W0"], np.float32); b0 = np.asarray(inputs["b0"], np.float32)
    W1 = np.asarray(inputs["W1"], np.float32); b1 = np.asarray(inputs["b1"], np.float32)
    W2 = np.asarray(inputs["W2"], np.float32); b2 = np.asarray(inputs["b2"], np.float32)
    G0 = np.asarray(inputs["G0"], np.float32); g0 = np.asarray(inputs["g0"], np.float32)
    G1 = np.asarray(inputs["G1"], np.float32); g1 = np.asarray(inputs["g1"], np.float32)

    expected = (points.shape == (N, 3) and num_clusters == C
                and cluster.shape == (N,))
    if expected:
        # contiguous equal clusters of 32 points, as produced by setup_inputs
        expected = bool(
            np.array_equal(cluster[::PTS], np.arange(C, dtype=np.int64))
            and np.array_equal(cluster, np.repeat(cluster[::PTS], PTS))
        )
    if not expected:
        return _numpy_fallback(points, cluster, num_clusters,
                               W0, b0, W1, b1, W2, b2, G0, g0, G1, g1)

    from concourse.bass_utils import run_bass_kernel_spmd

    if "nc" not in _CACHE:
        _CACHE["nc"] = _build_module(N4C)
    nc = _CACHE["nc"]

    in_maps = _host_pack(points, W0, b0, W1, b1, W2, b2, G0, g0, G1, g1, N4C)
    res = run_bass_kernel_spmd(nc, in_maps, core_ids=list(range(NCORES)))
    outs = []
    for c in range(NCORES):
        o = np.asarray(res.results[c]["outt"]).astype(np.float32)  # [256, CPC]
        outs.append(o.T)                                           # [CPC, 256]
    return np.ascontiguousarray(np.concatenate(outs, axis=0))



# revision 22
# speedup vs baseline: 1.0347x; 1.0121x over previous
"""Trainium2 Bass kernel for a PointNet-style neighborhood encoder.

Computation (matches the reference nn.Module):
    h = relu(relu(relu(points @ W0 + b0) @ W1 + b1) @ W2 + b2)   # [N,3] -> [N,128]
    pooled = segment_max(h, cluster)                             # [C,128], 32 pts/cluster
    out = relu(relu(pooled @ G0 + g0) @ G1 + g1)                 # [C,256]

Sharding: data-parallel over points across 8 NeuronCores (cluster
boundaries are shard-aligned because clusters are contiguous, 32
points each). Weights are replicated. No collectives; the host
scatters inputs and gathers per-core outputs.

Device strategy (per core, n = 262144 points = 65536 quad-columns):
  - Host packs points feature-major, 4 points per 128-partition column
    ("quads"): pts4[3a+f, q] = points[4q+a, f], so layer 0 is a single
    block-diagonal matmul (K=12, M=128) producing h0 for 4 points/col.
  - Layer 1 uses two permuted block-diagonal stationaries W1A/W1B
    (K=128, M=128) producing h1 with 2 points per column.
  - Layer 2 uses W2 duplicated on both partition halves; 4 sub-matmuls
    (K=64, M=128) with rhs partition slices map to distinct PE row
    groups, producing z = W2^T h1 (bias/relu deferred) in PSUM.
  - segment_max: relu is monotone and b2 is constant per feature, so
    pooled = relu(max_p(z) + b2). max over (4 tensors x 8 quads) is ONE
    VectorE tensor_reduce(axis=XY) straight out of PSUM per sub-chunk.
  - ScalarE (ACT) does every relu+bias PSUM->SBUF evacuation; VectorE
    only does the pooling reduces. bf16 activations everywhere
    (PSUM stays f32 as the HW requires).
  - Global MLP on pooled [128, 8192] per core; output is written
    feature-major [256, 8192] bf16 and transposed/upcast on the host.
"""

import numpy as np

# ---- problem geometry (hardcoded per contract) ----
N = 2097152          # total points
C = 65536            # clusters
PTS = 32             # points per cluster
NCORES = 8
NPC = N // NCORES    # points per core = 262144
N4C = NPC // 4       # quad-columns per core = 65536
CPC = C // NCORES    # clusters per core = 8192

BIG = 1024           # quad-columns per big-chunk
SUB = 256            # quad-columns per L2/pool sub-chunk
NCHUNK = N4C // BIG  # 64
NSUB = BIG // SUB    # 4

_CACHE = {}


def _bf16():
    import ml_dtypes
    return ml_dtypes.bfloat16


def _build_module(n4c: int):
    """Build the Bass module (SPMD program, same for all cores).

    Engine assignment per steady-state iteration (chunk of 1024 quad-cols
    = 4096 points = 128 clusters).  Hardware legality constraints (the
    real walrus birverifier, stricter than CoreSim): at most one PSUM
    operand per instruction, Pool/gpsimd has no tensor_tensor and no PSUM
    access, DMA cannot read PSUM.
      - PE: L0 for chunk i+1 (emitted one iteration early so the
        L0->h0s->L1a chain spans an iteration boundary), L1a/L1b for
        chunk i, L2 fills for chunk i-1 (16 mm into 4 PSUM tiles zp_j
        laid out (t4, c32, q8)), plus one G matmul on a 4-iter cadence.
      - ACT: the three PSUM->SBUF relu+bias evacuations (h1as, h0s-next,
        h1bs; one 1024-col instruction each), the raw bf16 copy of zp3
        (route C), and the G-chain gout evacuations.
      - DVE: tensor_reduce (t,q)->cluster straight from PSUM for
        zp0..zp2 (route R), the 5-level bf16 2x tensor_tensor max tree
        over the copied zp3, and the g1in evacuation.
      - Pool (gpsimd): only g0in = relu(pooled + b2) (SBUF tensor_scalar).
    All maxes commute; bf16 rounding is monotone so round-then-max ==
    max-then-round, and relu/bias commute with max (applied post-pool).
    """
    import concourse.bass as bass
    import concourse.bacc as bacc
    import concourse.tile as tile
    from concourse import mybir

    BF = mybir.dt.bfloat16
    F32 = mybir.dt.float32
    RELU = mybir.ActivationFunctionType.Relu
    MAX = mybir.AluOpType.max
    ADD = mybir.AluOpType.add
    XY = mybir.AxisListType.XY
    COPY = mybir.ActivationFunctionType.Copy

    nchunk = n4c // BIG
    cpc = n4c // 8          # clusters per core for this size
    nblk = cpc // 512       # global-MLP blocks of 512 clusters

    nc = bacc.Bacc()

    # ---- DRAM I/O ----
    pts4 = nc.dram_tensor("pts4", [12, n4c], BF, kind="ExternalInput")
    w0q = nc.dram_tensor("w0q", [12, 128], BF, kind="ExternalInput")
    w1a = nc.dram_tensor("w1a", [128, 128], BF, kind="ExternalInput")
    w1b = nc.dram_tensor("w1b", [128, 128], BF, kind="ExternalInput")
    w2d = nc.dram_tensor("w2d", [128, 128], BF, kind="ExternalInput")
    g0w = nc.dram_tensor("g0w", [128, 128], BF, kind="ExternalInput")
    g1lo = nc.dram_tensor("g1lo", [128, 128], BF, kind="ExternalInput")
    g1hi = nc.dram_tensor("g1hi", [128, 128], BF, kind="ExternalInput")
    b0q = nc.dram_tensor("b0q", [128, 1], F32, kind="ExternalInput")
    b1d = nc.dram_tensor("b1d", [128, 1], F32, kind="ExternalInput")
    b2v = nc.dram_tensor("b2v", [128, 1], F32, kind="ExternalInput")
    g0v = nc.dram_tensor("g0v", [128, 1], F32, kind="ExternalInput")
    g1l = nc.dram_tensor("g1l", [128, 1], F32, kind="ExternalInput")
    g1h = nc.dram_tensor("g1h", [128, 1], F32, kind="ExternalInput")
    outt = nc.dram_tensor("outt", [256, cpc], BF, kind="ExternalOutput")

    from contextlib import ExitStack
    with tile.TileContext(nc) as tc, ExitStack() as ctx:
        singles = ctx.enter_context(tc.tile_pool(name="singles", bufs=1))
        ppts = ctx.enter_context(tc.tile_pool(name="ppts", bufs=3))
        ph0s = ctx.enter_context(tc.tile_pool(name="ph0s", bufs=2))
        ph1s = ctx.enter_context(tc.tile_pool(name="ph1s", bufs=4))
        pzc = ctx.enter_context(tc.tile_pool(name="pzc", bufs=2))
        pu1 = ctx.enter_context(tc.tile_pool(name="pu1", bufs=2))
        pu2 = ctx.enter_context(tc.tile_pool(name="pu2", bufs=2))
        pu3 = ctx.enter_context(tc.tile_pool(name="pu3", bufs=2))
        pu4 = ctx.enter_context(tc.tile_pool(name="pu4", bufs=2))
        pg0 = ctx.enter_context(tc.tile_pool(name="pg0", bufs=2))
        pg1 = ctx.enter_context(tc.tile_pool(name="pg1", bufs=2))
        pgo = ctx.enter_context(tc.tile_pool(name="pgo", bufs=4))
        # PSUM: phx (2 bufs x 1024 f32 = 4 banks) rotates h0p -> [gp] ->
        # h1p -> h1p2; pz (2 bufs x 1024 = 4 banks) rotates the 4 zp tiles.
        phx = ctx.enter_context(tc.tile_pool(name="phx", bufs=2, space="PSUM"))
        pz = ctx.enter_context(tc.tile_pool(name="pz", bufs=2, space="PSUM"))

        # ---- load constants ----
        w0q_s = singles.tile([12, 128], BF)
        w1a_s = singles.tile([128, 128], BF)
        w1b_s = singles.tile([128, 128], BF)
        w2d_s = singles.tile([128, 128], BF)
        g0w_s = singles.tile([128, 128], BF)
        g1lo_s = singles.tile([128, 128], BF)
        g1hi_s = singles.tile([128, 128], BF)
        b0q_s = singles.tile([128, 1], F32)
        b1d_s = singles.tile([128, 1], F32)
        b2v_s = singles.tile([128, 1], F32)
        g0v_s = singles.tile([128, 1], F32)
        g1l_s = singles.tile([128, 1], F32)
        g1h_s = singles.tile([128, 1], F32)
        # load order matters for the prologue: the first h0s/h1 evacs wait
        # on b0q/b1d, and the first L0/L1/L2 matmuls on w0q/w1a/w1b/w2d;
        # the global-MLP constants aren't needed until ~10 iterations in.
        for dst, src in ((w0q_s, w0q), (b0q_s, b0q), (b1d_s, b1d),
                         (w1a_s, w1a), (w1b_s, w1b), (w2d_s, w2d),
                         (b2v_s, b2v), (g0v_s, g0v), (g1l_s, g1l),
                         (g1h_s, g1h), (g0w_s, g0w), (g1lo_s, g1lo),
                         (g1hi_s, g1hi)):
            nc.gpsimd.dma_start(out=dst[:], in_=src[:])

        # dependency-free dummy activation: pulls the one-time 1283ns
        # ACT table load off the first h0s evacuation's critical path
        warm = singles.tile([128, 1], F32)
        nc.gpsimd.memset(warm[:], 0.0)
        warm2 = singles.tile([128, 1], F32)
        nc.scalar.activation(warm2[:], warm[:], RELU, bias=0.0)

        # pooled raw max(z) accumulator (pre-bias/relu), bf16
        pooled = singles.tile([128, cpc], BF)

        # ---- helpers ----
        st = {}       # chunk -> {"h1as","h1bs","s_all","t1","t2"}
        gst = {}      # block -> {"g0in","g1in"}

        def fills(c, j, zp, ts):
            """L2 matmuls (subset `ts` of the 4 t-slots) for sub-chunk j of
            chunk c into zp: layout (t4, c32, q8) per 256-col t-slot."""
            s0 = j * SUB
            h1as = st[c]["h1as"]; h1bs = st[c]["h1bs"]
            for t in ts:
                src, half = ((h1as, 0), (h1bs, 0), (h1as, 1), (h1bs, 1))[t]
                nc.tensor.matmul(zp[:, t * SUB:(t + 1) * SUB],
                                 w2d_s[half * 64:half * 64 + 64, :],
                                 src[half * 64:half * 64 + 64, s0:s0 + SUB])

        def reduce_zp(c, j, zp):
            """DVE tensor_reduce (t,q) straight out of PSUM -> 32 pooled."""
            zv = zp.rearrange("p (t c q) -> p c t q", t=4, q=8)
            base = c * 128 + j * 32
            nc.vector.tensor_reduce(pooled[:, base:base + 32], zv[:],
                                    axis=XY, op=MAX)

        # G-phase schedule: block k phases 0..3 at iterations 4k+6+p
        # (block k's last pooled cols land in iteration 4k+5: chunk 4k+3's
        # C-route tree runs at c1 = it-2)
        gph = {}
        for k in range(nblk):
            for p in range(4):
                gph[4 * k + 9 + p] = (p, k)

        # Iteration `it` emits: L1a/L1b + h1 evacs for chunk it, L0 + h0s
        # for chunk it+1 (one iteration early, so the L0->h0s->L1a chain
        # spans an iteration boundary instead of serializing inside one),
        # L2 fills+merges for it-1, tree T1 for it-2, T2 for it-3, T3+T4
        # for it-4, and one G phase.
        for it in range(-1, nchunk + 9):
            mc = it if 0 <= it < nchunk else None       # main chunk
            lc = it + 1 if it + 1 < nchunk else None    # L0 chunk
            cf = it - 1 if 1 <= it - 1 + 1 <= nchunk else None
            c1 = it - 2 if 0 <= it - 2 < nchunk else None
            c2 = it - 3 if 0 <= it - 3 < nchunk else None
            c3 = it - 4 if 0 <= it - 4 < nchunk else None
            g = gph.get(it)

            # -- PE pos 1: L1a(mc) + ACT h1as --
            if mc is not None:
                h0s = st[mc]["h0s"]
                h1p = phx.tile([128, BIG], F32, tag="hx")
                nc.tensor.matmul(h1p[:, 0:512], w1a_s[:], h0s[:, 0:512])
                nc.tensor.matmul(h1p[:, 512:1024], w1a_s[:], h0s[:, 512:1024])
                h1as = ph1s.tile([128, BIG], BF, tag="h1as")
                nc.scalar.activation(h1as[:], h1p[:], RELU, bias=b1d_s[:])
                st[mc]["h1as"] = h1as

            # -- PE pos 2: L0(lc) + ACT h0s --
            if lc is not None:
                pts_t = ppts.tile([12, BIG], BF, tag="pts")
                nc.sync.dma_start(out=pts_t[:], in_=pts4[:, lc * BIG:(lc + 1) * BIG])
                h0p = phx.tile([128, BIG], F32, tag="hx")
                nc.tensor.matmul(h0p[:, 0:512], w0q_s[:], pts_t[:, 0:512])
                nc.tensor.matmul(h0p[:, 512:1024], w0q_s[:], pts_t[:, 512:1024])
                h0s_n = ph0s.tile([128, BIG], BF, tag="h0s")
                nc.scalar.activation(h0s_n[:], h0p[:], RELU, bias=b0q_s[:])
                st[lc] = {"h0s": h0s_n}

            # -- PE pos 3/4: fills j0, j1 + DVE reduces --
            if cf is not None:
                zp0 = pz.tile([128, BIG], F32, tag="zp")
                fills(cf, 0, zp0, (0, 2, 1, 3))
                zp1 = pz.tile([128, BIG], F32, tag="zp")
                fills(cf, 1, zp1, (0, 2, 1, 3))
                reduce_zp(cf, 0, zp0)
                reduce_zp(cf, 1, zp1)

            # -- PE pos 5: G matmul (phases 1..3) + Pool evac --
            if g is not None and g[0] >= 1:
                p, k = g
                sl = slice(k * 512, (k + 1) * 512)
                gp = phx.tile([128, 512], F32, tag="hx")
                if p == 1:
                    nc.tensor.matmul(gp[:], g0w_s[:], gst[k]["g0in"][:])
                    g1in = pg1.tile([128, 512], BF, tag="g1in")
                    if k % 2 == 0:
                        nc.vector.tensor_scalar(out=g1in[:], in0=gp[:],
                                                scalar1=g0v_s[:], scalar2=0.0,
                                                op0=ADD, op1=MAX)
                    else:
                        nc.scalar.activation(g1in[:], gp[:], RELU,
                                             bias=g0v_s[:])
                    gst[k]["g1in"] = g1in
                else:
                    wsrc, bsrc, base = ((g1lo_s, g1l_s, 0),
                                        (g1hi_s, g1h_s, 128))[p - 2]
                    nc.tensor.matmul(gp[:], wsrc[:], gst[k]["g1in"][:])
                    go = pgo.tile([128, 512], BF, tag="gout")
                    nc.scalar.activation(go[:], gp[:], RELU, bias=bsrc[:])
                    nc.sync.dma_start(out=outt[base:base + 128, sl], in_=go[:])
            else:
                # parity keeper: unused phx acquisition so the 4-slot
                # rotation pairing stays identical on non-Gmm iterations
                gp = phx.tile([128, 512], F32, tag="hx")

            # -- PE pos 7: L1b(mc) + ACT h1bs --
            if mc is not None:
                h1p2 = phx.tile([128, BIG], F32, tag="hx")
                nc.tensor.matmul(h1p2[:, 0:512], w1b_s[:], h0s[:, 0:512])
                nc.tensor.matmul(h1p2[:, 512:1024], w1b_s[:], h0s[:, 512:1024])
                h1bs = ph1s.tile([128, BIG], BF, tag="h1bs")
                nc.scalar.activation(h1bs[:], h1p2[:], RELU, bias=b1d_s[:])
                st[mc]["h1bs"] = h1bs

            # -- PE pos 6: fills j2 + reduce --
            if cf is not None:
                zp2 = pz.tile([128, BIG], F32, tag="zp")
                fills(cf, 2, zp2, (0, 2, 1, 3))
                reduce_zp(cf, 2, zp2)

            # -- PE pos 8: fills j3; route R (every 4th chunk) or C --
            if cf is not None:
                zp3 = pz.tile([128, BIG], F32, tag="zp")
                fills(cf, 3, zp3, (0, 2, 1, 3))
                if False:
                    reduce_zp(cf, 3, zp3)
                else:
                    zc = pzc.tile([128, BIG], BF, tag="zc")
                    nc.scalar.activation(zc[:], zp3[:], COPY, bias=0.0)
                    st[cf]["zc"] = zc

            # -- DVE tail: C-route tt-max tree for chunk c1's zp3 copy --
            if c1 is not None:
                zc = st[c1]["zc"]
                u1 = pu1.tile([128, 512], BF, tag="u1")
                nc.vector.tensor_tensor(u1[:], zc[:, 0:512], zc[:, 512:1024],
                                        op=MAX)
                u2 = pu2.tile([128, 256], BF, tag="u2")
                nc.vector.tensor_tensor(u2[:], u1[:, 0:256], u1[:, 256:512],
                                        op=MAX)
                u3 = pu3.tile([128, 128], BF, tag="u3")
                v3 = u2.rearrange("p (c two q) -> p c two q", two=2, q=4)
                nc.vector.tensor_tensor(
                    u3.rearrange("p (c q) -> p c q", q=4)[:],
                    v3[:, :, 0, :], v3[:, :, 1, :], op=MAX)
                u4 = pu4.tile([128, 64], BF, tag="u4")
                v4 = u3.rearrange("p (c two q) -> p c two q", two=2, q=2)
                nc.vector.tensor_tensor(
                    u4.rearrange("p (c q) -> p c q", q=2)[:],
                    v4[:, :, 0, :], v4[:, :, 1, :], op=MAX)
                v5 = u4.rearrange("p (c two) -> p c two", two=2)
                nc.vector.tensor_tensor(
                    pooled[:, c1 * 128 + 96:c1 * 128 + 128]
                    .rearrange("p (c one) -> p c one", one=1)[:],
                    v5[:, :, 0:1], v5[:, :, 1:2], op=MAX)
            if g is not None and g[0] == 0:
                k = g[1]
                g0in = pg0.tile([128, 512], BF, tag="g0in")
                nc.gpsimd.tensor_scalar(out=g0in[:],
                                        in0=pooled[:, k * 512:(k + 1) * 512],
                                        scalar1=b2v_s[:], scalar2=0.0,
                                        op0=ADD, op1=MAX)
                gst[k] = {"g0in": g0in}

    nc.compile()
    return nc


def _host_pack(points, W0, b0, W1, b1, W2, b2, G0, g0, G1, g1, n4c):
    """Build per-core input maps (host-side layout prep, numpy only)."""
    bf16 = _bf16()
    n = n4c * 4 * NCORES

    # pts4[3a+f, q] = points[4q+a, f]
    pts4 = np.ascontiguousarray(
        points[:n].reshape(-1, 4, 3).transpose(1, 2, 0).reshape(12, -1)
    ).astype(bf16)

    # W0 block-diagonal over 4 points: [12, 128]
    w0q = np.zeros((12, 128), np.float32)
    for a in range(4):
        w0q[3 * a:3 * a + 3, 32 * a:32 * a + 32] = W0
    # W1A/W1B: rows 32a+f; cols 64a'+g ; a' in {0,1} / {2,3}
    w1a = np.zeros((128, 128), np.float32)
    w1b = np.zeros((128, 128), np.float32)
    for a in range(2):
        w1a[32 * a:32 * a + 32, 64 * a:64 * a + 64] = W1
        w1b[32 * (a + 2):32 * (a + 2) + 32, 64 * a:64 * a + 64] = W1
    # W2 duplicated on both partition halves
    w2d = np.concatenate([W2, W2], axis=0)

    common = {
        "w0q": w0q.astype(bf16),
        "w1a": w1a.astype(bf16),
        "w1b": w1b.astype(bf16),
        "w2d": w2d.astype(bf16),
        "g0w": G0.astype(bf16),
        "g1lo": G1[:, :128].astype(bf16),
        "g1hi": G1[:, 128:].astype(bf16),
        "b0q": np.tile(b0, 4).reshape(128, 1).astype(np.float32),
        "b1d": np.tile(b1, 2).reshape(128, 1).astype(np.float32),
        "b2v": b2.reshape(128, 1).astype(np.float32),
        "g0v": g0.reshape(128, 1).astype(np.float32),
        "g1l": g1[:128].reshape(128, 1).astype(np.float32),
        "g1h": g1[128:].reshape(128, 1).astype(np.float32),
    }
    in_maps = []
    for c in range(NCORES):
        m = dict(common)
        m["pts4"] = np.ascontiguousarray(pts4[:, c * n4c:(c + 1) * n4c])
        in_maps.append(m)
    return in_maps


def _numpy_fallback(points, cluster, num_clusters,
                    W0, b0, W1, b1, W2, b2, G0, g0, G1, g1):
    h = points.astype(np.float32)
    for W, b in ((W0, b0), (W1, b1), (W2, b2)):
        h = np.maximum(h @ W + b, 0.0)
    order = np.argsort(cluster, kind="stable")
    cs = cluster[order]
    hs = h[order]
    starts = np.searchsorted(cs, np.arange(num_clusters), side="left")
    counts = np.bincount(cs, minlength=num_clusters)
    safe_starts = np.minimum(starts, max(len(hs) - 1, 0))
    seg = np.maximum.reduceat(hs, safe_starts, axis=0)
    seg[counts == 0] = -np.inf   # match segment_max identity on empties
    pooled = seg
    gx = pooled
    for W, b in ((G0, g0), (G1, g1)):
        gx = np.maximum(gx @ W + b, 0.0)
    return gx.astype(np.float32)


def kernel(**inputs) -> np.ndarray:
    points = np.asarray(inputs["points"], np.float32)
    cluster = np.asarray(inputs["cluster"]).astype(np.int64)
    num_clusters = int(np.asarray(inputs["num_clusters"]))
    W0 = np.asarray(inputs["W0"], np.float32); b0 = np.asarray(inputs["b0"], np.float32)
    W1 = np.asarray(inputs["W1"], np.float32); b1 = np.asarray(inputs["b1"], np.float32)
    W2 = np.asarray(inputs["W2"], np.float32); b2 = np.asarray(inputs["b2"], np.float32)
    G0 = np.asarray(inputs["G0"], np.float32); g0 = np.asarray(inputs["g0"], np.float32)
    G1 = np.asarray(inputs["G1"], np.float32); g1 = np.asarray(inputs["g1"], np.float32)

    expected = (points.shape == (N, 3) and num_clusters == C
                and cluster.shape == (N,))
    if expected:
        # contiguous equal clusters of 32 points, as produced by setup_inputs
        expected = bool(
            np.array_equal(cluster[::PTS], np.arange(C, dtype=np.int64))
            and np.array_equal(cluster, np.repeat(cluster[::PTS], PTS))
        )
    if not expected:
        return _numpy_fallback(points, cluster, num_clusters,
                               W0, b0, W1, b1, W2, b2, G0, g0, G1, g1)

    from concourse.bass_utils import run_bass_kernel_spmd

    if "nc" not in _CACHE:
        _CACHE["nc"] = _build_module(N4C)
    nc = _CACHE["nc"]

    in_maps = _host_pack(points, W0, b0, W1, b1, W2, b2, G0, g0, G1, g1, N4C)
    res = run_bass_kernel_spmd(nc, in_maps, core_ids=list(range(NCORES)))
    outs = []
    for c in range(NCORES):
        o = np.asarray(res.results[c]["outt"]).astype(np.float32)  # [256, CPC]
        outs.append(o.T)                                           # [CPC, 256]
    return np.ascontiguousarray(np.concatenate(outs, axis=0))



# revision 26
# speedup vs baseline: 1.0395x; 1.0046x over previous
"""Trainium2 Bass kernel for a PointNet-style neighborhood encoder.

Computation (matches the reference nn.Module):
    h = relu(relu(relu(points @ W0 + b0) @ W1 + b1) @ W2 + b2)   # [N,3] -> [N,128]
    pooled = segment_max(h, cluster)                             # [C,128], 32 pts/cluster
    out = relu(relu(pooled @ G0 + g0) @ G1 + g1)                 # [C,256]

Sharding: data-parallel over points across 8 NeuronCores (cluster
boundaries are shard-aligned because clusters are contiguous, 32
points each). Weights are replicated. No collectives; the host
scatters inputs and gathers per-core outputs.

Device strategy (per core, n = 262144 points = 65536 quad-columns):
  - Host packs points feature-major, 4 points per 128-partition column
    ("quads"): pts4[3a+f, q] = points[4q+a, f], so layer 0 is a single
    block-diagonal matmul (K=12, M=128) producing h0 for 4 points/col.
  - Layer 1 uses two permuted block-diagonal stationaries W1A/W1B
    (K=128, M=128) producing h1 with 2 points per column.
  - Layer 2 uses W2 duplicated on both partition halves; 4 sub-matmuls
    (K=64, M=128) with rhs partition slices map to distinct PE row
    groups, producing z = W2^T h1 (bias/relu deferred) in PSUM.
  - segment_max: relu is monotone and b2 is constant per feature, so
    pooled = relu(max_p(z) + b2). max over (4 tensors x 8 quads) is ONE
    VectorE tensor_reduce(axis=XY) straight out of PSUM per sub-chunk.
  - ScalarE (ACT) does every relu+bias PSUM->SBUF evacuation; VectorE
    only does the pooling reduces. bf16 activations everywhere
    (PSUM stays f32 as the HW requires).
  - Global MLP on pooled [128, 8192] per core; output is written
    feature-major [256, 8192] bf16 and transposed/upcast on the host.
"""

import numpy as np

# ---- problem geometry (hardcoded per contract) ----
N = 2097152          # total points
C = 65536            # clusters
PTS = 32             # points per cluster
NCORES = 8
NPC = N // NCORES    # points per core = 262144
N4C = NPC // 4       # quad-columns per core = 65536
CPC = C // NCORES    # clusters per core = 8192

BIG = 1024           # quad-columns per big-chunk
SUB = 256            # quad-columns per L2/pool sub-chunk
NCHUNK = N4C // BIG  # 64
NSUB = BIG // SUB    # 4

_CACHE = {}


def _bf16():
    import ml_dtypes
    return ml_dtypes.bfloat16


def _build_module(n4c: int):
    """Build the Bass module (SPMD program, same for all cores).

    Engine assignment per steady-state iteration (chunk of 1024 quad-cols
    = 4096 points = 128 clusters).  Hardware legality constraints (the
    real walrus birverifier, stricter than CoreSim): at most one PSUM
    operand per instruction, Pool/gpsimd has no tensor_tensor and no PSUM
    access, DMA cannot read PSUM.
      - PE: L0 for chunk i+1 (emitted one iteration early so the
        L0->h0s->L1a chain spans an iteration boundary), L1a/L1b for
        chunk i, L2 fills for chunk i-1 (16 mm into 4 PSUM tiles zp_j
        laid out (t4, c32, q8)), plus one G matmul on a 4-iter cadence.
      - ACT: the three PSUM->SBUF relu+bias evacuations (h1as, h0s-next,
        h1bs; one 1024-col instruction each), the raw bf16 copy of zp3
        (route C), and the G-chain gout evacuations.
      - DVE: tensor_reduce (t,q)->cluster straight from PSUM for
        zp0..zp2 (route R), the 5-level bf16 2x tensor_tensor max tree
        over the copied zp3, and the g1in evacuation.
      - Pool (gpsimd): only g0in = relu(pooled + b2) (SBUF tensor_scalar).
    All maxes commute; bf16 rounding is monotone so round-then-max ==
    max-then-round, and relu/bias commute with max (applied post-pool).
    """
    import concourse.bass as bass
    import concourse.bacc as bacc
    import concourse.tile as tile
    from concourse import mybir

    BF = mybir.dt.bfloat16
    F32 = mybir.dt.float32
    RELU = mybir.ActivationFunctionType.Relu
    MAX = mybir.AluOpType.max
    ADD = mybir.AluOpType.add
    XY = mybir.AxisListType.XY
    COPY = mybir.ActivationFunctionType.Copy

    nchunk = n4c // BIG
    cpc = n4c // 8          # clusters per core for this size
    nblk = cpc // 512       # global-MLP blocks of 512 clusters

    nc = bacc.Bacc()

    # ---- DRAM I/O ----
    pts4 = nc.dram_tensor("pts4", [12, n4c], BF, kind="ExternalInput")
    w0q = nc.dram_tensor("w0q", [12, 128], BF, kind="ExternalInput")
    w1a = nc.dram_tensor("w1a", [128, 128], BF, kind="ExternalInput")
    w1b = nc.dram_tensor("w1b", [128, 128], BF, kind="ExternalInput")
    w2d = nc.dram_tensor("w2d", [128, 128], BF, kind="ExternalInput")
    g0w = nc.dram_tensor("g0w", [128, 128], BF, kind="ExternalInput")
    g1lo = nc.dram_tensor("g1lo", [128, 128], BF, kind="ExternalInput")
    g1hi = nc.dram_tensor("g1hi", [128, 128], BF, kind="ExternalInput")
    b0q = nc.dram_tensor("b0q", [128, 1], F32, kind="ExternalInput")
    b1d = nc.dram_tensor("b1d", [128, 1], F32, kind="ExternalInput")
    b2v = nc.dram_tensor("b2v", [128, 1], F32, kind="ExternalInput")
    g0v = nc.dram_tensor("g0v", [128, 1], F32, kind="ExternalInput")
    g1l = nc.dram_tensor("g1l", [128, 1], F32, kind="ExternalInput")
    g1h = nc.dram_tensor("g1h", [128, 1], F32, kind="ExternalInput")
    outt = nc.dram_tensor("outt", [256, cpc], BF, kind="ExternalOutput")

    from contextlib import ExitStack
    with tile.TileContext(nc) as tc, ExitStack() as ctx:
        singles = ctx.enter_context(tc.tile_pool(name="singles", bufs=1))
        ppts = ctx.enter_context(tc.tile_pool(name="ppts", bufs=3))
        ph0s = ctx.enter_context(tc.tile_pool(name="ph0s", bufs=2))
        ph1s = ctx.enter_context(tc.tile_pool(name="ph1s", bufs=4))
        pzc = ctx.enter_context(tc.tile_pool(name="pzc", bufs=2))
        pu1 = ctx.enter_context(tc.tile_pool(name="pu1", bufs=2))
        pu2 = ctx.enter_context(tc.tile_pool(name="pu2", bufs=2))
        pu3 = ctx.enter_context(tc.tile_pool(name="pu3", bufs=2))
        pu4 = ctx.enter_context(tc.tile_pool(name="pu4", bufs=2))
        pg0 = ctx.enter_context(tc.tile_pool(name="pg0", bufs=2))
        pg1 = ctx.enter_context(tc.tile_pool(name="pg1", bufs=2))
        pgo = ctx.enter_context(tc.tile_pool(name="pgo", bufs=4))
        # PSUM: phx (2 bufs x 1024 f32 = 4 banks) rotates h0p -> [gp] ->
        # h1p -> h1p2; pz (2 bufs x 1024 = 4 banks) rotates the 4 zp tiles.
        phx = ctx.enter_context(tc.tile_pool(name="phx", bufs=2, space="PSUM"))
        pz = ctx.enter_context(tc.tile_pool(name="pz", bufs=2, space="PSUM"))

        # ---- load constants ----
        w0q_s = singles.tile([12, 128], BF)
        w1a_s = singles.tile([128, 128], BF)
        w1b_s = singles.tile([128, 128], BF)
        w2d_s = singles.tile([128, 128], BF)
        g0w_s = singles.tile([128, 128], BF)
        g1lo_s = singles.tile([128, 128], BF)
        g1hi_s = singles.tile([128, 128], BF)
        b0q_s = singles.tile([128, 1], F32)
        b1d_s = singles.tile([128, 1], F32)
        b2v_s = singles.tile([128, 1], F32)
        g0v_s = singles.tile([128, 1], F32)
        g1l_s = singles.tile([128, 1], F32)
        g1h_s = singles.tile([128, 1], F32)
        # load order matters for the prologue: the first h0s/h1 evacs wait
        # on b0q/b1d, and the first L0/L1/L2 matmuls on w0q/w1a/w1b/w2d;
        # the global-MLP constants aren't needed until ~10 iterations in.
        for dst, src in ((w0q_s, w0q), (b0q_s, b0q), (b1d_s, b1d),
                         (w1a_s, w1a), (w1b_s, w1b), (w2d_s, w2d),
                         (b2v_s, b2v), (g0v_s, g0v), (g1l_s, g1l),
                         (g1h_s, g1h), (g0w_s, g0w), (g1lo_s, g1lo),
                         (g1hi_s, g1hi)):
            nc.gpsimd.dma_start(out=dst[:], in_=src[:])

        # dependency-free dummy activation: pulls the one-time 1283ns
        # ACT table load off the first h0s evacuation's critical path
        warm = singles.tile([128, 1], F32)
        nc.gpsimd.memset(warm[:], 0.0)
        warm2 = singles.tile([128, 1], F32)
        nc.scalar.activation(warm2[:], warm[:], RELU, bias=0.0)

        # pooled raw max(z) accumulator (pre-bias/relu), bf16
        pooled = singles.tile([128, cpc], BF)

        # ---- helpers ----
        st = {}       # chunk -> {"h1as","h1bs","s_all","t1","t2"}
        gst = {}      # block -> {"g0in","g1in"}

        def fills(c, j, zp, ts):
            """L2 matmuls (subset `ts` of the 4 t-slots) for sub-chunk j of
            chunk c into zp: layout (t4, c32, q8) per 256-col t-slot."""
            s0 = j * SUB
            h1as = st[c]["h1as"]; h1bs = st[c]["h1bs"]
            for t in ts:
                src, half = ((h1as, 0), (h1bs, 0), (h1as, 1), (h1bs, 1))[t]
                nc.tensor.matmul(zp[:, t * SUB:(t + 1) * SUB],
                                 w2d_s[half * 64:half * 64 + 64, :],
                                 src[half * 64:half * 64 + 64, s0:s0 + SUB])

        def reduce_zp(c, j, zp):
            """DVE tensor_reduce (t,q) straight out of PSUM -> 32 pooled."""
            zv = zp.rearrange("p (t c q) -> p c t q", t=4, q=8)
            base = c * 128 + j * 32
            nc.vector.tensor_reduce(pooled[:, base:base + 32], zv[:],
                                    axis=XY, op=MAX)

        # G-phase schedule: block k phases 0..3 at iterations 4k+6+p
        # (block k's last pooled cols land in iteration 4k+5: chunk 4k+3's
        # C-route tree runs at c1 = it-2)
        gph = {}
        for k in range(nblk):
            for p in range(4):
                gph[4 * k + 9 + p] = (p, k)

        # Iteration `it` emits: L1a/L1b + h1 evacs for chunk it, L0 + h0s
        # for chunk it+1 (one iteration early, so the L0->h0s->L1a chain
        # spans an iteration boundary instead of serializing inside one),
        # L2 fills+merges for it-1, tree T1 for it-2, T2 for it-3, T3+T4
        # for it-4, and one G phase.
        for it in range(-1, nchunk + 9):
            mc = it if 0 <= it < nchunk else None       # main chunk
            lc = it + 1 if it + 1 < nchunk else None    # L0 chunk
            cf = it - 1 if 1 <= it - 1 + 1 <= nchunk else None
            c1 = it - 2 if 0 <= it - 2 < nchunk else None
            c2 = it - 3 if 0 <= it - 3 < nchunk else None
            c3 = it - 4 if 0 <= it - 4 < nchunk else None
            g = gph.get(it)

            # -- PE pos 2: fills j0 + DVE reduce (feeds DVE earliest) --
            if cf is not None:
                zp0 = pz.tile([128, BIG], F32, tag="zp")
                fills(cf, 0, zp0, (0, 2, 1, 3))
                reduce_zp(cf, 0, zp0)

            # -- PE pos 1: L1a(mc) + ACT h1as --
            if mc is not None:
                h0s = st[mc]["h0s"]
                h1p = phx.tile([128, BIG], F32, tag="hx")
                nc.tensor.matmul(h1p[:, 0:512], w1a_s[:], h0s[:, 0:512])
                nc.tensor.matmul(h1p[:, 512:1024], w1a_s[:], h0s[:, 512:1024])
                h1as = ph1s.tile([128, BIG], BF, tag="h1as")
                nc.scalar.activation(h1as[:], h1p[:], RELU, bias=b1d_s[:])
                st[mc]["h1as"] = h1as

            # -- PE pos 3: L0(lc) + ACT h0s --
            if lc is not None:
                pts_t = ppts.tile([12, BIG], BF, tag="pts")
                nc.sync.dma_start(out=pts_t[:], in_=pts4[:, lc * BIG:(lc + 1) * BIG])
                h0p = phx.tile([128, BIG], F32, tag="hx")
                nc.tensor.matmul(h0p[:, 0:512], w0q_s[:], pts_t[:, 0:512])
                nc.tensor.matmul(h0p[:, 512:1024], w0q_s[:], pts_t[:, 512:1024])
                h0s_n = ph0s.tile([128, BIG], BF, tag="h0s")
                nc.scalar.activation(h0s_n[:], h0p[:], RELU, bias=b0q_s[:])
                st[lc] = {"h0s": h0s_n}

            # -- PE pos 4: fills j1 + DVE reduce --
            if cf is not None:
                zp1 = pz.tile([128, BIG], F32, tag="zp")
                fills(cf, 1, zp1, (0, 2, 1, 3))
                reduce_zp(cf, 1, zp1)


            # -- PE pos 5: G matmul (phases 1..3) + Pool evac --
            if g is not None and g[0] >= 1:
                p, k = g
                sl = slice(k * 512, (k + 1) * 512)
                gp = phx.tile([128, 512], F32, tag="hx")
                if p == 1:
                    nc.tensor.matmul(gp[:], g0w_s[:], gst[k]["g0in"][:])
                    g1in = pg1.tile([128, 512], BF, tag="g1in")
                    if k % 2 == 0:
                        nc.vector.tensor_scalar(out=g1in[:], in0=gp[:],
                                                scalar1=g0v_s[:], scalar2=0.0,
                                                op0=ADD, op1=MAX)
                    else:
                        nc.scalar.activation(g1in[:], gp[:], RELU,
                                             bias=g0v_s[:])
                    gst[k]["g1in"] = g1in
                else:
                    wsrc, bsrc, base = ((g1lo_s, g1l_s, 0),
                                        (g1hi_s, g1h_s, 128))[p - 2]
                    nc.tensor.matmul(gp[:], wsrc[:], gst[k]["g1in"][:])
                    go = pgo.tile([128, 512], BF, tag="gout")
                    nc.scalar.activation(go[:], gp[:], RELU, bias=bsrc[:])
                    nc.sync.dma_start(out=outt[base:base + 128, sl], in_=go[:])
            else:
                # parity keeper: unused phx acquisition so the 4-slot
                # rotation pairing stays identical on non-Gmm iterations
                gp = phx.tile([128, 512], F32, tag="hx")

            # -- PE pos 7: L1b(mc) + ACT h1bs --
            if mc is not None:
                h1p2 = phx.tile([128, BIG], F32, tag="hx")
                nc.tensor.matmul(h1p2[:, 0:512], w1b_s[:], h0s[:, 0:512])
                nc.tensor.matmul(h1p2[:, 512:1024], w1b_s[:], h0s[:, 512:1024])
                h1bs = ph1s.tile([128, BIG], BF, tag="h1bs")
                nc.scalar.activation(h1bs[:], h1p2[:], RELU, bias=b1d_s[:])
                st[mc]["h1bs"] = h1bs

            # -- PE pos 6: fills j2 + reduce --
            if cf is not None:
                zp2 = pz.tile([128, BIG], F32, tag="zp")
                fills(cf, 2, zp2, (0, 2, 1, 3))
                reduce_zp(cf, 2, zp2)

            # -- PE pos 8: fills j3; route R (every 4th chunk) or C --
            if cf is not None:
                zp3 = pz.tile([128, BIG], F32, tag="zp")
                fills(cf, 3, zp3, (0, 2, 1, 3))
                if False:
                    reduce_zp(cf, 3, zp3)
                else:
                    zc = pzc.tile([128, BIG], BF, tag="zc")
                    nc.scalar.activation(zc[:], zp3[:], COPY, bias=0.0)
                    st[cf]["zc"] = zc

            # -- DVE tail: C-route tt-max tree for chunk c1's zp3 copy --
            if c1 is not None:
                zc = st[c1]["zc"]
                u1 = pu1.tile([128, 512], BF, tag="u1")
                nc.vector.tensor_tensor(u1[:], zc[:, 0:512], zc[:, 512:1024],
                                        op=MAX)
                u2 = pu2.tile([128, 256], BF, tag="u2")
                nc.vector.tensor_tensor(u2[:], u1[:, 0:256], u1[:, 256:512],
                                        op=MAX)
                u3 = pu3.tile([128, 128], BF, tag="u3")
                v3 = u2.rearrange("p (c two q) -> p c two q", two=2, q=4)
                nc.vector.tensor_tensor(
                    u3.rearrange("p (c q) -> p c q", q=4)[:],
                    v3[:, :, 0, :], v3[:, :, 1, :], op=MAX)
                u4 = pu4.tile([128, 64], BF, tag="u4")
                v4 = u3.rearrange("p (c two q) -> p c two q", two=2, q=2)
                nc.vector.tensor_tensor(
                    u4.rearrange("p (c q) -> p c q", q=2)[:],
                    v4[:, :, 0, :], v4[:, :, 1, :], op=MAX)
                v5 = u4.rearrange("p (c two) -> p c two", two=2)
                nc.vector.tensor_tensor(
                    pooled[:, c1 * 128 + 96:c1 * 128 + 128]
                    .rearrange("p (c one) -> p c one", one=1)[:],
                    v5[:, :, 0:1], v5[:, :, 1:2], op=MAX)
            if g is not None and g[0] == 0:
                k = g[1]
                g0in = pg0.tile([128, 512], BF, tag="g0in")
                nc.gpsimd.tensor_scalar(out=g0in[:],
                                        in0=pooled[:, k * 512:(k + 1) * 512],
                                        scalar1=b2v_s[:], scalar2=0.0,
                                        op0=ADD, op1=MAX)
                gst[k] = {"g0in": g0in}

    nc.compile()
    return nc


def _host_pack(points, W0, b0, W1, b1, W2, b2, G0, g0, G1, g1, n4c):
    """Build per-core input maps (host-side layout prep, numpy only)."""
    bf16 = _bf16()
    n = n4c * 4 * NCORES

    # pts4[3a+f, q] = points[4q+a, f]
    pts4 = np.ascontiguousarray(
        points[:n].reshape(-1, 4, 3).transpose(1, 2, 0).reshape(12, -1)
    ).astype(bf16)

    # W0 block-diagonal over 4 points: [12, 128]
    w0q = np.zeros((12, 128), np.float32)
    for a in range(4):
        w0q[3 * a:3 * a + 3, 32 * a:32 * a + 32] = W0
    # W1A/W1B: rows 32a+f; cols 64a'+g ; a' in {0,1} / {2,3}
    w1a = np.zeros((128, 128), np.float32)
    w1b = np.zeros((128, 128), np.float32)
    for a in range(2):
        w1a[32 * a:32 * a + 32, 64 * a:64 * a + 64] = W1
        w1b[32 * (a + 2):32 * (a + 2) + 32, 64 * a:64 * a + 64] = W1
    # W2 duplicated on both partition halves
    w2d = np.concatenate([W2, W2], axis=0)

    common = {
        "w0q": w0q.astype(bf16),
        "w1a": w1a.astype(bf16),
        "w1b": w1b.astype(bf16),
        "w2d": w2d.astype(bf16),
        "g0w": G0.astype(bf16),
        "g1lo": G1[:, :128].astype(bf16),
        "g1hi": G1[:, 128:].astype(bf16),
        "b0q": np.tile(b0, 4).reshape(128, 1).astype(np.float32),
        "b1d": np.tile(b1, 2).reshape(128, 1).astype(np.float32),
        "b2v": b2.reshape(128, 1).astype(np.float32),
        "g0v": g0.reshape(128, 1).astype(np.float32),
        "g1l": g1[:128].reshape(128, 1).astype(np.float32),
        "g1h": g1[128:].reshape(128, 1).astype(np.float32),
    }
    in_maps = []
    for c in range(NCORES):
        m = dict(common)
        m["pts4"] = np.ascontiguousarray(pts4[:, c * n4c:(c + 1) * n4c])
        in_maps.append(m)
    return in_maps


def _numpy_fallback(points, cluster, num_clusters,
                    W0, b0, W1, b1, W2, b2, G0, g0, G1, g1):
    h = points.astype(np.float32)
    for W, b in ((W0, b0), (W1, b1), (W2, b2)):
        h = np.maximum(h @ W + b, 0.0)
    order = np.argsort(cluster, kind="stable")
    cs = cluster[order]
    hs = h[order]
    starts = np.searchsorted(cs, np.arange(num_clusters), side="left")
    counts = np.bincount(cs, minlength=num_clusters)
    safe_starts = np.minimum(starts, max(len(hs) - 1, 0))
    seg = np.maximum.reduceat(hs, safe_starts, axis=0)
    seg[counts == 0] = -np.inf   # match segment_max identity on empties
    pooled = seg
    gx = pooled
    for W, b in ((G0, g0), (G1, g1)):
        gx = np.maximum(gx @ W + b, 0.0)
    return gx.astype(np.float32)


def kernel(**inputs) -> np.ndarray:
    points = np.asarray(inputs["points"], np.float32)
    cluster = np.asarray(inputs["cluster"]).astype(np.int64)
    num_clusters = int(np.asarray(inputs["num_clusters"]))
    W0 = np.asarray(inputs["W0"], np.float32); b0 = np.asarray(inputs["b0"], np.float32)
    W1 = np.asarray(inputs["W1"], np.float32); b1 = np.asarray(inputs["b1"], np.float32)
    W2 = np.asarray(inputs["W2"], np.float32); b2 = np.asarray(inputs["b2"], np.float32)
    G0 = np.asarray(inputs["G0"], np.float32); g0 = np.asarray(inputs["g0"], np.float32)
    G1 = np.asarray(inputs["G1"], np.float32); g1 = np.asarray(inputs["g1"], np.float32)

    expected = (points.shape == (N, 3) and num_clusters == C
                and cluster.shape == (N,))
    if expected:
        # contiguous equal clusters of 32 points, as produced by setup_inputs
        expected = bool(
            np.array_equal(cluster[::PTS], np.arange(C, dtype=np.int64))
            and np.array_equal(cluster, np.repeat(cluster[::PTS], PTS))
        )
    if not expected:
        return _numpy_fallback(points, cluster, num_clusters,
                               W0, b0, W1, b1, W2, b2, G0, g0, G1, g1)

    from concourse.bass_utils import run_bass_kernel_spmd

    if "nc" not in _CACHE:
        _CACHE["nc"] = _build_module(N4C)
    nc = _CACHE["nc"]

    in_maps = _host_pack(points, W0, b0, W1, b1, W2, b2, G0, g0, G1, g1, N4C)
    res = run_bass_kernel_spmd(nc, in_maps, core_ids=list(range(NCORES)))
    outs = []
    for c in range(NCORES):
        o = np.asarray(res.results[c]["outt"]).astype(np.float32)  # [256, CPC]
        outs.append(o.T)                                           # [CPC, 256]
    return np.ascontiguousarray(np.concatenate(outs, axis=0))

